# revision 1
# baseline (speedup 1.0000x reference)
"""Trainium2 Bass kernel for MultiLayerCrossModalAttention (v2).

Contract: kernel(**inputs) takes the FULL fp32 inputs (as in
reference.setup_inputs) and returns the FULL [B,C,H,W] fp32 output.
Sharding: core = b*2 + half (batch x H-halves) over 8 NeuronCores; the
white/K/V side carries a 4-pixel (1-token) halo so the 3x3 token-neighbor
attention needs no cross-core traffic.

Device layout: C=128 on SBUF partitions, pixel grid on the free dim.
The per-channel token attention runs on the pixel grid:
  S_n = blocksum_4x4(Q * shift_n(K)); softmax over the 9 shifts;
  O = sum_n upsample(A_n) * shift_n(V).
This execution environment charges a large fixed cost per instruction
(~40us DVE/PE, ~90us ScalarE), so the kernel is built around very few,
maximal-size instructions: 3-neighbor-fused multiplies via access-pattern
stride surgery, single-instruction block-sum reductions (reduce XY over a
5-D view), gpsimd partition_all_reduce for the LayerNorm channel sums
(replacing 32 matmuls/layer), and a single activation-table set.
"""

import os
import sys

import numpy as np

if "/opt/trn_rl_repo" not in sys.path:
    sys.path.insert(0, "/opt/trn_rl_repo")

import ml_dtypes

TS = 4
C = 128
NUM_LAYERS = 2
SCALE = float((TS * TS) ** -0.5)  # 0.25
LN_EPS = 1e-5

B, H, W = 4, 128, 128
ROWS = H // 2          # 64 rows per core
KROWS = ROWS + 2 * TS  # 72 rows incl halo
PW = W + 2 * TS        # 136 padded cols
NTH = ROWS // TS       # 16 token rows per core
NTW = W // TS          # 32 token cols
NTOK = NTH * NTW       # 512 tokens per core
NPIX = ROWS * W        # 8192 pixels per core

PCOLS = 3 * C + 8      # packed param columns (w_qT|w_kT|w_vT|6 vectors|pad)

_CACHE = {}


def _restride(ap, dim, step):
    """Return a copy of `ap` with free-dim `dim`'s step replaced."""
    b = ap.copy()
    b.ap[dim] = [step, b.ap[dim][1]]
    return b


def _build(reps=1, masked=True):
    import concourse.bass as bass
    import concourse.tile as tile
    from concourse import bacc, bass_isa, mybir

    # Pin all used activations (exp/ln/square/copy/identity) to the one
    # table set containing them all, so only one ACT_TABLE_LOAD is emitted.
    if not getattr(bacc, "_act_tables_patched", False):
        _orig_tables = bacc.get_activation_tables
        _KEEP = "natural_log_exp_and_others"

        def _patched(arch):
            t = _orig_tables(arch)
            mine = t[_KEEP]
            return {
                name: (fns if name == _KEEP else (fns - mine))
                for name, fns in t.items()
            }

        bacc.get_activation_tables = _patched
        bacc._act_tables_patched = True

    F32 = mybir.dt.float32
    BF16 = mybir.dt.bfloat16
    AX = mybir.AxisListType
    ALU = mybir.AluOpType
    ACTF = mybir.ActivationFunctionType
    RED = bass_isa.ReduceOp

    nc = bacc.Bacc("TRN2", target_bir_lowering=False, debug=False, num_devices=8)

    d_blue = nc.dram_tensor("blue", [C, ROWS, W], F32, kind="ExternalInput").ap()
    d_white = nc.dram_tensor("white", [C, KROWS, W], F32,
                             kind="ExternalInput").ap()
    d_par = nc.dram_tensor("par", [NUM_LAYERS, C, PCOLS], F32,
                           kind="ExternalInput").ap()
    d_consts = nc.dram_tensor("consts", [C, 3], F32, kind="ExternalInput").ap()
    d_out = nc.dram_tensor("out", [C, ROWS, W], F32, kind="ExternalOutput").ap()

    with tile.TileContext(nc) as tc:
        with (
            nc.allow_low_precision("bf16 compute by design"),
            tc.tile_pool(name="pp", bufs=1) as pp,
            tc.tile_pool(name="psp", bufs=1, space="PSUM") as psp,
        ):
            acc = pp.tile([C, ROWS, W], F32)          # 32K: blue -> out
            cb = pp.tile([C, ROWS, W], F32)           # 32K: current_blue
            Kt = pp.tile([C, KROWS, PW], BF16)        # 19K padded K
            Vt = pp.tile([C, KROWS, PW], BF16)        # 19K padded V
            S = pp.tile([C, 9, NTOK], BF16)           # 9K logits/attn
            den = pp.tile([C, NTOK], F32)             # 2K
            Odi = pp.tile([C, NPIX], BF16)            # 16K combine partial
            consts = pp.tile([C, 3], F32)
            eps_t = pp.tile([C, 1], F32)

            nc.sync.dma_start(cb[:], d_blue[:])
            nc.sync.dma_start(consts[:], d_consts[:])
            nc.vector.memset(eps_t[:], LN_EPS)
            nc.vector.tensor_scalar_add(
                acc[:].rearrange("c h w -> c (h w)"),
                cb[:].rearrange("c h w -> c (h w)"), consts[:, 2:3])
            mtop = consts[:, 0:1]
            mbot = consts[:, 1:2]
            # zero the x-margins of Kt/Vt once (convs never write them)
            for t in (Kt, Vt):
                m = _restride(
                    t[:, :, 0:TS].unsqueeze(2).broadcast_to(
                        [C, KROWS, 2, TS]), 2, W + TS)
                nc.gpsimd.memset(m, 0.0)

            with tc.tile_pool(name="lp", bufs=1) as lp:
              for rep in range(reps):
                for li in range(NUM_LAYERS):
                    if True:
                        par = lp.tile([C, PCOLS], F32, tag="par")
                        nc.sync.dma_start(par[:], d_par[li])
                        wq = par[:, 0:C]
                        wk = par[:, C:2 * C]
                        wv = par[:, 2 * C:3 * C]
                        qb = par[:, 3 * C + 0:3 * C + 1]
                        kb = par[:, 3 * C + 1:3 * C + 2]
                        vb = par[:, 3 * C + 2:3 * C + 3]
                        g_ = par[:, 3 * C + 3:3 * C + 4]
                        lwg = par[:, 3 * C + 4:3 * C + 5]

                        # big shared slot: white -> P3 -> tmp3 -> LN tiles
                        def big():
                            return lp.tile([C, 3, NPIX], BF16, tag="big", name="bigslot")

                        # QO slot: Qt during logits, O during combine/LN
                        def qo():
                            return lp.tile([C, ROWS, W], BF16, tag="qo", name="qoslot")

                        axp = lp.tile([C, 3, NTH, NTW, TS], BF16, tag="axp")

                        # ---- white + convs (K, V on 72 halo rows; Q on cb)
                        wt_full = big()
                        white = wt_full[:].rearrange(
                            "c a b -> c (a b)").bitcast(F32)[
                            :, 0:KROWS * W].rearrange(
                            "c (h w) -> c h w", w=W)
                        nc.sync.dma_start(white, d_white[:])

                        Qt = qo()
                        wf = white.rearrange("c h w -> c (h w)")
                        cbf = cb[:].rearrange("c h w -> c (h w)")
                        # grouped per weight so LDWEIGHTS is loaded once each
                        for (wmat, bias, npx, dst) in (
                            (wk, kb, KROWS * W, Kt),
                            (wv, vb, KROWS * W, Vt),
                            (wq, qb, NPIX, None),
                        ):
                            src = cbf if dst is None else wf
                            px0 = 0
                            while px0 < npx:
                                px1 = min(px0 + 4096, npx)
                                ps = psp.tile([C, 4096], F32, tag="ps")
                                for k in range(px0, px1, 512):
                                    nc.tensor.matmul(
                                        ps[:, k - px0:k - px0 + 512],
                                        wmat, src[:, k:k + 512],
                                        start=True, stop=True)
                                n = px1 - px0
                                if dst is None:
                                    o = Qt[:].rearrange(
                                        "c h w -> c (h w)")[:, px0:px1]
                                else:
                                    o = dst[:, px0 // W:px1 // W, TS:TS + W]
                                nc.vector.tensor_scalar_add(
                                    o, ps[:, 0:n].rearrange(
                                        "c (h w) -> c h w", w=W)
                                    if dst is not None else ps[:, 0:n],
                                    bias)
                                px0 = px1

                        # zero out-of-image halo rows (global top/bottom);
                        # with all-zero conv biases the zero-padded white
                        # already yields zero K/V there, so this is skipped
                        if masked:
                            for t in (Kt, Vt):
                                nc.vector.tensor_scalar_mul(
                                    t[:, 0:TS, :], t[:, 0:TS, :], mtop)
                                nc.vector.tensor_scalar_mul(
                                    t[:, ROWS + TS:KROWS, :],
                                    t[:, ROWS + TS:KROWS, :], mbot)

                        # ---- logits: S[3di:3di+3] = blocksum(Q * shift(K))
                        for di in range(3):
                            P3 = big()
                            qb3 = Qt[:].unsqueeze(1).broadcast_to(
                                [C, 3, ROWS, W])
                            kb3 = _restride(
                                Kt[:, 4 * di:4 * di + ROWS, 0:W]
                                .unsqueeze(1).broadcast_to([C, 3, ROWS, W]),
                                1, TS)
                            p3v = P3[:].rearrange("c n (h w) -> c n h w", w=W)
                            nc.vector.tensor_mul(p3v, qb3, kb3)
                            v = p3v.rearrange(
                                "c dj (th r) (tw s) -> c dj th tw r s",
                                r=TS, s=TS).rearrange(
                                "c dj th tw r s -> c (dj th) tw r s")
                            nc.vector.reduce_sum(
                                S[:, 3 * di:3 * di + 3, :], v, axis=AX.XY)

                        # ---- softmax over 9 neighbors (logits are O(0.3),
                        # no max-subtraction; OOB neighbors get logit 0 and
                        # V=0, matching the reference's zero-padded K/V)
                        sf = S[:].rearrange("c n t -> c (n t)")
                        nc.scalar.activation(sf, sf, ACTF.Exp, scale=SCALE)
                        nc.vector.reduce_sum(
                            den[:], S[:].rearrange("c n t -> c t n"),
                            axis=AX.X)
                        nc.vector.reciprocal(den[:], den[:])

                        # ---- combine: O = sum_n upsample(A_n) * shift_n(V)
                        O = qo()
                        rden_b = den[:].rearrange(
                            "c (th tw) -> c th tw", tw=NTW).unsqueeze(
                            1).unsqueeze(4).broadcast_to(
                            [C, 3, NTH, NTW, TS])
                        for di in range(3):
                            nc.vector.tensor_mul(
                                axp[:],
                                S[:, 3 * di:3 * di + 3, :].rearrange(
                                    "c n (th tw) -> c n th tw", tw=NTW)
                                .unsqueeze(4).broadcast_to(
                                    [C, 3, NTH, NTW, TS]),
                                rden_b)
                            tmp3 = big()
                            t3v = tmp3[:].rearrange(
                                "c n (th r x) -> c n th r x", r=TS, x=W)
                            vap = _restride(
                                Vt[:, 4 * di:4 * di + ROWS, 0:W]
                                .unsqueeze(1).broadcast_to([C, 3, ROWS, W]),
                                1, TS).rearrange(
                                "c n (th r) x -> c n th r x", r=TS)
                            aap = axp[:].rearrange(
                                "c n th tw s -> c n th (tw s)").unsqueeze(
                                3).broadcast_to([C, 3, NTH, TS, W])
                            nc.vector.tensor_mul(t3v, vap, aap)
                            dst = O if di == 0 else Odi
                            nc.vector.reduce_sum(
                                dst[:].rearrange("c h w -> c (h w)")
                                if di == 0 else dst[:],
                                tmp3[:].rearrange("c n p -> c p n"),
                                axis=AX.X)
                            if di > 0:
                                nc.vector.tensor_add(
                                    O[:].rearrange("c h w -> c (h w)"),
                                    O[:].rearrange("c h w -> c (h w)"),
                                    Odi[:])

                        # ---- LayerNorm over C (gpsimd channel all-reduce)
                        ln = big()
                        Of = O[:].rearrange("c h w -> c (h w)")
                        sq, s1, s2 = ln[:, 0, :], ln[:, 1, :], ln[:, 2, :]
                        nc.vector.tensor_mul(sq, Of, Of)
                        nc.gpsimd.partition_all_reduce(
                            s1, Of, channels=C, reduce_op=RED.add)
                        nc.gpsimd.partition_all_reduce(
                            s2, sq, channels=C, reduce_op=RED.add)
                        # s1 <- -mu ; sq <- mu^2 ; s2 <- var -> istd
                        nc.vector.tensor_scalar_mul(s1, s1, -1.0 / C)
                        nc.vector.tensor_mul(sq, s1, s1)
                        nc.vector.scalar_tensor_tensor(
                            s2, s2, 1.0 / C, sq,
                            op0=ALU.mult, op1=ALU.subtract)
                        nc.scalar.activation(s2, s2, ACTF.Ln, bias=eps_t[:])
                        nc.scalar.activation(s2, s2, ACTF.Exp, scale=-0.5)
                        # O <- ((O - mu) * istd) * g + b
                        nc.vector.tensor_add(Of, Of, s1)
                        nc.vector.tensor_mul(Of, Of, s2)
                        accf = acc[:].rearrange("c h w -> c (h w)")
                        nc.vector.scalar_tensor_tensor(
                            accf, Of, lwg, accf, op0=ALU.mult, op1=ALU.add)
                        if li == 0:
                            nc.vector.scalar_tensor_tensor(
                                cbf, Of, g_, cbf, op0=ALU.mult, op1=ALU.add)

            nc.sync.dma_start(d_out[:], acc[:])

    nc.compile()
    return nc


def _prep_inputs(blue, white, q_w, q_b, k_w, k_b, v_w, v_b, ln_g, ln_b,
                 layer_weights):
    bf16 = ml_dtypes.bfloat16
    f32 = np.float32

    whiteP = np.zeros((B, C, H + 2 * TS, W), dtype=f32)
    whiteP[:, :, TS:TS + H, :] = np.asarray(white, f32)

    q_w = np.asarray(q_w, f32)
    q_b = np.asarray(q_b, f32)
    ln_b = np.asarray(ln_b, f32)
    ln_g = np.asarray(ln_g, f32)
    lwv = np.asarray(layer_weights, f32)
    par = np.zeros((NUM_LAYERS, C, PCOLS), dtype=f32)
    par[:, :, 0:C] = np.transpose(q_w, (0, 2, 1))
    par[:, :, C:2 * C] = np.transpose(np.asarray(k_w, f32), (0, 2, 1))
    par[:, :, 2 * C:3 * C] = np.transpose(np.asarray(v_w, f32), (0, 2, 1))
    # current_blue tracked on device WITHOUT ln_b[0] (enh0 = g0*t2 + b0);
    # fold the missing b0 into layer-2's Q bias: W2 @ (cb + b0) + q_b2
    par[0, :, 3 * C + 0] = q_b[0]
    par[1, :, 3 * C + 0] = q_b[1] + q_w[1] @ ln_b[0]
    par[:, :, 3 * C + 1] = np.asarray(k_b, f32)
    par[:, :, 3 * C + 2] = np.asarray(v_b, f32)
    par[:, :, 3 * C + 3] = ln_g
    par[:, :, 3 * C + 4] = ln_g * lwv.reshape(NUM_LAYERS, 1)

    blue = np.asarray(blue, f32)
    in_maps = []
    for core in range(8):
        b, half = core // 2, core % 2
        y0 = half * ROWS
        consts = np.empty((C, 3), f32)
        consts[:, 0] = 0.0 if half == 0 else 1.0
        consts[:, 1] = 0.0 if half == 1 else 1.0
        consts[:, 2] = ln_b[0] * lwv[0] + ln_b[1] * lwv[1]
        in_maps.append({
            "blue": np.ascontiguousarray(blue[b, :, y0:y0 + ROWS, :]),
            "white": np.ascontiguousarray(whiteP[b, :, y0:y0 + KROWS, :]),
            "par": par,
            "consts": consts,
        })
    return in_maps


def kernel(**inputs):
    from concourse.bass_utils import run_bass_kernel_spmd

    reps = int(os.environ.get("KBENCH_REPS", "1"))
    masked = bool(
        np.any(np.asarray(inputs["k_b"])) or np.any(np.asarray(inputs["v_b"])))
    key = ("nc", reps, masked)
    if key not in _CACHE:
        _CACHE[key] = _build(reps, masked)
    nc = _CACHE[key]

    in_maps = _prep_inputs(**inputs)
    res = run_bass_kernel_spmd(nc, in_maps, core_ids=list(range(8)))

    out = np.empty((B, C, H, W), np.float32)
    for core in range(8):
        b, half = core // 2, core % 2
        y0 = half * ROWS
        out[b, :, y0:y0 + ROWS, :] = res.results[core]["out"]
    return out



# revision 2
# speedup vs baseline: 11.1693x; 11.1693x over previous
"""Trainium2 Bass kernel for MultiLayerCrossModalAttention (v4).

Contract: kernel(**inputs) takes FULL fp32 inputs, returns FULL [B,C,H,W]
fp32 output. Sharding: core = b*2 + half (batch x H-halves); the white/K/V
side carries a 4-pixel halo so attention needs no cross-core traffic.

v4 design (measured-cost driven):
- All convs in bf16 on PE, batched 8x512 into one [C,4096] PSUM tile
  (~0.25us/matmul), drained by ScalarE Identity(+bias) (~8us/4096).
- Software pipelined: layer li+1's K/V convs are emitted between layer
  li's combine and LayerNorm so PE/ACT conv work hides under DVE streams;
  drain order K,Q,V minimizes the DVE wait at the logits head.
- LayerNorm channel stats via ones-matmul on PE (replaces gpsimd
  partition_all_reduce, ~74us/op -> ~10us).
- current_blue eliminated: Q1 = Wq1@blue + (Wq1 diag(g0))@N0 folded into
  one PSUM accumulation group (host-side weight fold).
- attention combine: 9-term accumulation by wide bf16 adds (2x DVE mode)
  instead of 1x-mode reduces; softmax division folded into S per-slice.
- out = blue + acc is finished on HOST in f32 (device acc is bf16 enh sum).
"""

import os
import sys

import numpy as np

if "/opt/trn_rl_repo" not in sys.path:
    sys.path.insert(0, "/opt/trn_rl_repo")

import ml_dtypes

TS = 4
C = 128
NUM_LAYERS = 2
SCALE = float((TS * TS) ** -0.5)
LN_EPS = 1e-5

B, H, W = 4, 128, 128
ROWS = H // 2
KROWS = ROWS + 2 * TS
PW = W + 2 * TS
NTH = ROWS // TS
NTW = W // TS
NTOK = NTH * NTW
NPIX = ROWS * W

_CACHE = {}


def _restride(ap, dim, step):
    b = ap.copy()
    b.ap[dim] = [step, b.ap[dim][1]]
    return b


def _build(reps=1, masked=False):
    import contextlib
    import concourse.bass as bass
    import concourse.tile as tile
    from concourse import bacc, bass_isa, mybir

    if not getattr(bacc, "_act_tables_patched", False):
        _orig_tables = bacc.get_activation_tables
        _KEEP = "natural_log_exp_and_others"

        def _patched(arch):
            t = _orig_tables(arch)
            mine = t[_KEEP]
            return {
                name: (fns if name == _KEEP else (fns - mine))
                for name, fns in t.items()
            }

        bacc.get_activation_tables = _patched
        bacc._act_tables_patched = True

    F32 = mybir.dt.float32
    BF16 = mybir.dt.bfloat16
    AX = mybir.AxisListType
    ALU = mybir.AluOpType
    ACTF = mybir.ActivationFunctionType

    nc = bacc.Bacc("TRN2", target_bir_lowering=False, debug=False, num_devices=8)

    d_blueb = nc.dram_tensor("blueb", [C, NPIX], BF16, kind="ExternalInput").ap()
    d_whiteb = nc.dram_tensor("whiteb", [C, KROWS * W], BF16,
                              kind="ExternalInput").ap()
    d_w = nc.dram_tensor("w", [C, NUM_LAYERS * 4 * C], BF16,
                         kind="ExternalInput").ap()
    d_vecs = nc.dram_tensor("vecs", [C, NUM_LAYERS * 4], F32,
                            kind="ExternalInput").ap()
    d_consts = nc.dram_tensor("consts", [C, 4], F32, kind="ExternalInput").ap()
    d_out = nc.dram_tensor("out", [C, NPIX], BF16, kind="ExternalOutput").ap()

    with tile.TileContext(nc) as tc:
        with (
            nc.allow_low_precision("bf16 compute by design"),
            tc.tile_pool(name="pp", bufs=1) as pp,
            tc.tile_pool(name="psp", bufs=1, space="PSUM") as psp,
        ):
            acc = pp.tile([C, NPIX], BF16)        # 16K: weighted enh sum
            blueb = pp.tile([C, NPIX], BF16)      # 16K
            whiteb = pp.tile([C, KROWS * W], BF16)  # 18K
            Kt = pp.tile([C, KROWS, PW], BF16)    # 19.1K
            Vt = pp.tile([C, KROWS, PW], BF16)    # 19.1K
            T1 = pp.tile([C, NPIX], BF16)         # 16K: Qt
            T2 = pp.tile([C, NPIX], BF16)         # 16K: O / N (normalized)
            S = pp.tile([C, 9, NTOK], BF16)       # 9K
            den = pp.tile([C, NTOK], F32)         # 2K
            axp = pp.tile([C, 3, NTH, NTW, TS], BF16)  # 12K upsampled attn
            big = pp.tile([C, 3, NPIX], BF16)     # 48K: P3 / tmp3 / LN stats
            wts = pp.tile([C, NUM_LAYERS, 4 * C], BF16, name="wts")  # 2K
            vecs = pp.tile([C, NUM_LAYERS, 4], F32, name="vecs")
            consts = pp.tile([C, 4], F32)
            ones = pp.tile([C, C], BF16)
            eps_t = pp.tile([C, 1], F32)

            nc.sync.dma_start(blueb[:], d_blueb[:])
            nc.sync.dma_start(whiteb[:], d_whiteb[:])
            nc.sync.dma_start(wts[:], d_w[:])
            nc.sync.dma_start(vecs[:], d_vecs[:])
            nc.sync.dma_start(consts[:], d_consts[:])
            nc.vector.memset(eps_t[:], LN_EPS)
            nc.vector.memset(ones[:], 1.0)
            mtop = consts[:, 0:1]
            mbot = consts[:, 1:2]
            c0 = consts[:, 2:3]
            # zero x-margins of Kt/Vt once (drains never write them)
            for t in (Kt, Vt):
                m = _restride(
                    t[:, :, 0:TS].unsqueeze(2).broadcast_to(
                        [C, KROWS, 2, TS]), 2, W + TS)
                nc.gpsimd.memset(m, 0.0)

            ps = psp.tile([C, 4096], F32)

            def emit_conv(wmat, bias, dst, src, npx):
                """1x1 conv src->dst via PE + ACT Identity(+bias) drains."""
                px0 = 0
                while px0 < npx:
                    px1 = min(px0 + 4096, npx)
                    for k in range(px0, px1, 512):
                        nc.tensor.matmul(
                            ps[:, k - px0:k - px0 + 512],
                            wmat, src[:, k:k + 512], start=True, stop=True)
                    if dst is None:
                        o = T1[:, px0:px1]
                        i = ps[:, 0:px1 - px0]
                    else:
                        o = dst[:, px0 // W:px1 // W, TS:TS + W]
                        i = ps[:, 0:px1 - px0].rearrange("c (h w) -> c h w", w=W)
                    nc.scalar.activation(o, i, ACTF.Identity, bias=bias)
                    px0 = px1

            def emit_kv_conv(li, which):
                if which == "k":
                    wmat, bias, dst = wts[:, li, C:2 * C], vecs[:, li, 1:2], Kt
                else:
                    wmat, bias, dst = wts[:, li, 2 * C:3 * C], vecs[:, li, 2:3], Vt
                emit_conv(wmat, bias, dst, whiteb[:], KROWS * W)
                if masked:
                    nc.vector.tensor_scalar_mul(
                        dst[:, 0:TS, :], dst[:, 0:TS, :], mtop)
                    nc.vector.tensor_scalar_mul(
                        dst[:, ROWS + TS:KROWS, :],
                        dst[:, ROWS + TS:KROWS, :], mbot)

            def emit_q_conv(li):
                # li0: wq@blue; li1: wq@blue + wqg@N0 (PSUM accumulation)
                wq = wts[:, li, 0:C]
                wqg = wts[:, li, 3 * C:4 * C]
                qb = vecs[:, li, 0:1]
                for px0 in (0, 4096):
                    for k in range(px0, px0 + 4096, 512):
                        nc.tensor.matmul(
                            ps[:, k - px0:k - px0 + 512],
                            wq, blueb[:, k:k + 512],
                            start=True, stop=(li == 0))
                    if li == 1:
                        for k in range(px0, px0 + 4096, 512):
                            nc.tensor.matmul(
                                ps[:, k - px0:k - px0 + 512],
                                wqg, T2[:, k:k + 512],
                                start=False, stop=True,
                                skip_group_check=True)
                    nc.scalar.activation(
                        T1[:, px0:px0 + 4096], ps[:],
                        ACTF.Identity, bias=qb)

            # prologue: layer-0 convs (drain order K, Q, V: logits-di0
            # needs K rows 0:64 + Q; V only needed at combine)
            emit_kv_conv(0, "k")
            emit_q_conv(0)
            emit_kv_conv(0, "v")

            loop = tc.For_i(0, reps, 1) if reps > 1 else contextlib.nullcontext()
            with loop:
                for li in range(NUM_LAYERS):
                    lwg = vecs[:, li, 3:4]

                    # ---- logits: S[3di:3di+3] = blocksum(Q * shift(K))
                    Qv = T1[:].rearrange("c (h w) -> c h w", w=W)
                    for di in range(3):
                        qb3 = Qv.unsqueeze(1).broadcast_to([C, 3, ROWS, W])
                        kb3 = _restride(
                            Kt[:, 4 * di:4 * di + ROWS, 0:W]
                            .unsqueeze(1).broadcast_to([C, 3, ROWS, W]),
                            1, TS)
                        p3v = big[:].rearrange("c n (h w) -> c n h w", w=W)
                        nc.vector.tensor_mul(p3v, qb3, kb3)
                        v = p3v.rearrange(
                            "c dj (th r) (tw s) -> c dj th tw r s",
                            r=TS, s=TS).rearrange(
                            "c dj th tw r s -> c (dj th) tw r s")
                        nc.vector.reduce_sum(
                            S[:, 3 * di:3 * di + 3, :], v, axis=AX.XY)

                    # ---- softmax over 9 neighbors (logits O(0.3), no max)
                    sf = S[:].rearrange("c n t -> c (n t)")
                    nc.scalar.activation(sf, sf, ACTF.Exp, scale=SCALE)
                    nc.vector.reduce_sum(
                        den[:], S[:].rearrange("c n t -> c t n"), axis=AX.X)
                    nc.vector.reciprocal(den[:], den[:])
                    # divide S by den per-slice so upsample di=0 starts
                    # before the full S is scaled
                    db = den[:].unsqueeze(1)
                    nc.vector.tensor_mul(
                        S[:, 0:3], S[:, 0:3], db.broadcast_to([C, 3, NTOK]))
                    nc.vector.tensor_mul(
                        S[:, 3:9], S[:, 3:9], db.broadcast_to([C, 6, NTOK]))

                    # ---- combine: O = sum_n upsample(A_n) * shift_n(V)
                    O = T2[:]
                    for di in range(3):
                        src = S[:, 3 * di:3 * di + 3, :].rearrange(
                            "c n (th tw) -> c n th tw", tw=NTW).unsqueeze(
                            4).broadcast_to([C, 3, NTH, NTW, TS])
                        nc.scalar.copy(axp[:], src)
                        t3v = big[:].rearrange(
                            "c n (th r x) -> c n th r x", r=TS, x=W)
                        vap = _restride(
                            Vt[:, 4 * di:4 * di + ROWS, 0:W]
                            .unsqueeze(1).broadcast_to([C, 3, ROWS, W]),
                            1, TS).rearrange(
                            "c n (th r) x -> c n th r x", r=TS)
                        aap = axp[:].rearrange(
                            "c n th tw s -> c n th (tw s)").unsqueeze(
                            3).broadcast_to([C, 3, NTH, TS, W])
                        nc.vector.tensor_mul(t3v, vap, aap)
                        if di == 0:
                            nc.vector.tensor_add(O, big[:, 0, :], big[:, 1, :])
                            nc.vector.tensor_add(O, O, big[:, 2, :])
                        else:
                            for n in range(3):
                                nc.vector.tensor_add(O, O, big[:, n, :])

                    # ---- next layer's K conv: PE+ACT work that hides
                    # under this layer's LN DVE stream
                    nli = 1 - li
                    emit_next = (li == 0) or reps > 1
                    if emit_next:
                        emit_kv_conv(nli, "k")

                    # ---- LayerNorm over C via ones-matmul stats
                    o2 = big[:, 0, :]
                    mun = big[:, 1, :]
                    istd = big[:, 2, :]
                    nc.scalar.activation(o2, T2[:], ACTF.Square)
                    for (srcT, dstv, scl) in ((T2[:], mun, -1.0 / C),
                                              (o2, istd, 1.0 / C)):
                        for hx in (0, 4096):
                            for k in range(hx, hx + 4096, 512):
                                nc.tensor.matmul(
                                    ps[:, k - hx:k - hx + 512],
                                    ones[:], srcT[:, k:k + 512],
                                    start=True, stop=True)
                            nc.scalar.activation(
                                dstv[:, hx:hx + 4096], ps[:],
                                ACTF.Identity, scale=scl)
                    # istd holds E[x^2]; mun holds -mu
                    nc.scalar.activation(o2, mun, ACTF.Square)  # mu^2
                    nc.vector.tensor_add(T2[:], T2[:], mun)     # O - mu
                    nc.vector.tensor_sub(istd, istd, o2)        # var
                    nc.scalar.activation(istd, istd, ACTF.Ln, bias=eps_t[:])
                    nc.scalar.activation(istd, istd, ACTF.Exp, scale=-0.5)
                    if emit_next:
                        emit_kv_conv(nli, "v")
                    nc.vector.tensor_mul(T2[:], T2[:], istd)    # N
                    if li == 0:
                        nc.vector.tensor_scalar(
                            acc[:], T2[:], lwg, c0, op0=ALU.mult, op1=ALU.add)
                    else:
                        nc.vector.scalar_tensor_tensor(
                            acc[:], T2[:], lwg, acc[:],
                            op0=ALU.mult, op1=ALU.add)
                    if emit_next:
                        emit_q_conv(nli)

            nc.sync.dma_start(d_out[:], acc[:])

    nc.compile()
    return nc


def _prep_inputs(blue, white, q_w, q_b, k_w, k_b, v_w, v_b, ln_g, ln_b,
                 layer_weights):
    bf16 = ml_dtypes.bfloat16
    f32 = np.float32

    blue = np.asarray(blue, f32)
    whiteP = np.zeros((B, C, H + 2 * TS, W), dtype=f32)
    whiteP[:, :, TS:TS + H, :] = np.asarray(white, f32)

    q_w = np.asarray(q_w, f32)
    q_b = np.asarray(q_b, f32)
    k_w = np.asarray(k_w, f32)
    v_w = np.asarray(v_w, f32)
    ln_b = np.asarray(ln_b, f32)
    ln_g = np.asarray(ln_g, f32)
    lwv = np.asarray(layer_weights, f32)

    wpack = np.zeros((C, NUM_LAYERS, 4 * C), dtype=bf16)
    for li in range(NUM_LAYERS):
        wpack[:, li, 0:C] = q_w[li].T.astype(bf16)
        wpack[:, li, C:2 * C] = k_w[li].T.astype(bf16)
        wpack[:, li, 2 * C:3 * C] = v_w[li].T.astype(bf16)
    # Q1 = Wq1@blue + (Wq1 diag(g0))@N0  (+ qb1 + Wq1@b0)
    wpack[:, 1, 3 * C:4 * C] = (q_w[1].T * ln_g[0][:, None]).astype(bf16)

    vecs = np.zeros((C, NUM_LAYERS, 4), dtype=f32)
    vecs[:, 0, 0] = q_b[0]
    vecs[:, 1, 0] = q_b[1] + q_w[1] @ ln_b[0]
    vecs[:, :, 1] = np.asarray(k_b, f32).T
    vecs[:, :, 2] = np.asarray(v_b, f32).T
    vecs[:, :, 3] = (ln_g * lwv.reshape(NUM_LAYERS, 1)).T

    in_maps = []
    for core in range(8):
        b, half = core // 2, core % 2
        y0 = half * ROWS
        consts = np.zeros((C, 4), f32)
        consts[:, 0] = 0.0 if half == 0 else 1.0
        consts[:, 1] = 0.0 if half == 1 else 1.0
        consts[:, 2] = ln_b[0] * lwv[0] + ln_b[1] * lwv[1]
        in_maps.append({
            "blueb": np.ascontiguousarray(
                blue[b, :, y0:y0 + ROWS, :]).reshape(C, NPIX).astype(bf16),
            "whiteb": np.ascontiguousarray(
                whiteP[b, :, y0:y0 + KROWS, :]).reshape(
                C, KROWS * W).astype(bf16),
            "w": wpack.reshape(C, NUM_LAYERS * 4 * C),
            "vecs": vecs.reshape(C, NUM_LAYERS * 4),
            "consts": consts,
        })
    return in_maps


def kernel(**inputs):
    from concourse.bass_utils import run_bass_kernel_spmd

    reps = int(os.environ.get("KBENCH_REPS", "1"))
    masked = bool(
        np.any(np.asarray(inputs["k_b"])) or np.any(np.asarray(inputs["v_b"])))
    key = ("nc", reps, masked)
    if key not in _CACHE:
        _CACHE[key] = _build(reps, masked)
    nc = _CACHE[key]

    in_maps = _prep_inputs(**inputs)
    res = run_bass_kernel_spmd(nc, in_maps, core_ids=list(range(8)))

    blue = np.asarray(inputs["blue"], np.float32)
    out = np.empty((B, C, H, W), np.float32)
    for core in range(8):
        b, half = core // 2, core % 2
        y0 = half * ROWS
        enh = np.asarray(res.results[core]["out"],
                         np.float32).reshape(C, ROWS, W)
        out[b, :, y0:y0 + ROWS, :] = blue[b, :, y0:y0 + ROWS, :] + enh
    return out


# revision 3
# speedup vs baseline: 24.7502x; 2.2159x over previous
"""Trainium2 Bass kernel for MultiLayerCrossModalAttention (v4).

Contract: kernel(**inputs) takes FULL fp32 inputs, returns FULL [B,C,H,W]
fp32 output. Sharding: core = b*2 + half (batch x H-halves); the white/K/V
side carries a 4-pixel halo so attention needs no cross-core traffic.

v4 design (measured-cost driven):
- All convs in bf16 on PE, batched 8x512 into one [C,4096] PSUM tile
  (~0.25us/matmul), drained by ScalarE Identity(+bias) (~8us/4096).
- Software pipelined: layer li+1's K/V convs are emitted between layer
  li's combine and LayerNorm so PE/ACT conv work hides under DVE streams;
  drain order K,Q,V minimizes the DVE wait at the logits head.
- LayerNorm channel stats via ones-matmul on PE (replaces gpsimd
  partition_all_reduce, ~74us/op -> ~10us).
- current_blue eliminated: Q1 = Wq1@blue + (Wq1 diag(g0))@N0 folded into
  one PSUM accumulation group (host-side weight fold).
- attention combine: 9-term accumulation by wide bf16 adds (2x DVE mode)
  instead of 1x-mode reduces; softmax division folded into S per-slice.
- out = blue + acc is finished on HOST in f32 (device acc is bf16 enh sum).
"""

import os
import sys

import numpy as np

if "/opt/trn_rl_repo" not in sys.path:
    sys.path.insert(0, "/opt/trn_rl_repo")

import ml_dtypes

TS = 4
C = 128
NUM_LAYERS = 2
SCALE = float((TS * TS) ** -0.5)
LN_EPS = 1e-5

B, H, W = 4, 128, 128
ROWS = H // 2
KROWS = ROWS + 2 * TS
PW = W + 2 * TS
NTH = ROWS // TS
NTW = W // TS
NTOK = NTH * NTW
NPIX = ROWS * W

_CACHE = {}


def _restride(ap, dim, step):
    b = ap.copy()
    b.ap[dim] = [step, b.ap[dim][1]]
    return b


def _build(reps=1, masked=False):
    import contextlib
    import concourse.bass as bass
    import concourse.tile as tile
    from concourse import bacc, bass_isa, mybir

    if not getattr(bacc, "_act_tables_patched", False):
        _orig_tables = bacc.get_activation_tables
        _KEEP = "natural_log_exp_and_others"

        def _patched(arch):
            t = _orig_tables(arch)
            mine = t[_KEEP]
            return {
                name: (fns if name == _KEEP else (fns - mine))
                for name, fns in t.items()
            }

        bacc.get_activation_tables = _patched
        bacc._act_tables_patched = True

    F32 = mybir.dt.float32
    BF16 = mybir.dt.bfloat16
    AX = mybir.AxisListType
    ALU = mybir.AluOpType
    ACTF = mybir.ActivationFunctionType

    nc = bacc.Bacc("TRN2", target_bir_lowering=False, debug=False, num_devices=8)

    d_blueb = nc.dram_tensor("blueb", [C, NPIX], BF16, kind="ExternalInput").ap()
    d_whiteb = nc.dram_tensor("whiteb", [C, KROWS * W], BF16,
                              kind="ExternalInput").ap()
    d_w = nc.dram_tensor("w", [C, NUM_LAYERS * 4 * C], BF16,
                         kind="ExternalInput").ap()
    d_vecs = nc.dram_tensor("vecs", [C, NUM_LAYERS * 4], F32,
                            kind="ExternalInput").ap()
    d_consts = nc.dram_tensor("consts", [C, 4], F32, kind="ExternalInput").ap()
    d_out = nc.dram_tensor("out", [C, NPIX], BF16, kind="ExternalOutput").ap()

    with tile.TileContext(nc) as tc:
        with (
            nc.allow_low_precision("bf16 compute by design"),
            tc.tile_pool(name="pp", bufs=1) as pp,
            tc.tile_pool(name="psp", bufs=1, space="PSUM") as psp,
        ):
            acc = pp.tile([C, NPIX], BF16)        # 16K: weighted enh sum
            blueb = pp.tile([C, NPIX], BF16)      # 16K
            whiteb = pp.tile([C, KROWS * W], BF16)  # 18K
            Kt = pp.tile([C, KROWS, PW], BF16)    # 19.1K
            Vt = pp.tile([C, KROWS, PW], BF16)    # 19.1K
            T1 = pp.tile([C, NPIX], BF16)         # 16K: Qt
            T2 = pp.tile([C, NPIX], BF16)         # 16K: O / N (normalized)
            S = pp.tile([C, 9, NTOK], BF16)       # 9K
            den = pp.tile([C, NTOK], F32)         # 2K
            axp = pp.tile([C, 3, NTH, NTW, TS], BF16)  # 12K upsampled attn
            big = pp.tile([C, 3, NPIX], BF16)     # 48K: P3 / tmp3 / LN stats
            wts = pp.tile([C, NUM_LAYERS, 4 * C], BF16, name="wts")  # 2K
            vecs = pp.tile([C, NUM_LAYERS, 4], F32, name="vecs")
            consts = pp.tile([C, 4], F32)
            ones = pp.tile([C, C], BF16)
            eps_t = pp.tile([C, 1], F32)

            nc.sync.dma_start(blueb[:], d_blueb[:])
            nc.sync.dma_start(whiteb[:], d_whiteb[:])
            nc.sync.dma_start(wts[:], d_w[:])
            nc.sync.dma_start(vecs[:], d_vecs[:])
            nc.sync.dma_start(consts[:], d_consts[:])
            nc.vector.memset(eps_t[:], LN_EPS)
            nc.vector.memset(ones[:], 1.0)
            mtop = consts[:, 0:1]
            mbot = consts[:, 1:2]
            c0 = consts[:, 2:3]
            # zero x-margins of Kt/Vt once (drains never write them)
            for t in (Kt, Vt):
                m = _restride(
                    t[:, :, 0:TS].unsqueeze(2).broadcast_to(
                        [C, KROWS, 2, TS]), 2, W + TS)
                nc.gpsimd.memset(m, 0.0)

            ps = psp.tile([C, 4096], F32)

            def emit_conv(wmat, bias, dst, src, npx):
                """1x1 conv src->dst via PE + ACT Identity(+bias) drains."""
                px0 = 0
                while px0 < npx:
                    px1 = min(px0 + 4096, npx)
                    for k in range(px0, px1, 512):
                        nc.tensor.matmul(
                            ps[:, k - px0:k - px0 + 512],
                            wmat, src[:, k:k + 512], start=True, stop=True)
                    if dst is None:
                        o = T1[:, px0:px1]
                        i = ps[:, 0:px1 - px0]
                    else:
                        o = dst[:, px0 // W:px1 // W, TS:TS + W]
                        i = ps[:, 0:px1 - px0].rearrange("c (h w) -> c h w", w=W)
                    nc.scalar.activation(o, i, ACTF.Identity, bias=bias)
                    px0 = px1

            def emit_kv_conv(li, which):
                if which == "k":
                    wmat, bias, dst = wts[:, li, C:2 * C], vecs[:, li, 1:2], Kt
                else:
                    wmat, bias, dst = wts[:, li, 2 * C:3 * C], vecs[:, li, 2:3], Vt
                emit_conv(wmat, bias, dst, whiteb[:], KROWS * W)
                if masked:
                    nc.vector.tensor_scalar_mul(
                        dst[:, 0:TS, :], dst[:, 0:TS, :], mtop)
                    nc.vector.tensor_scalar_mul(
                        dst[:, ROWS + TS:KROWS, :],
                        dst[:, ROWS + TS:KROWS, :], mbot)

            def emit_q_conv(li):
                # li0: wq@blue; li1: wq@blue + wqg@N0 (PSUM accumulation)
                wq = wts[:, li, 0:C]
                wqg = wts[:, li, 3 * C:4 * C]
                qb = vecs[:, li, 0:1]
                for px0 in (0, 4096):
                    for k in range(px0, px0 + 4096, 512):
                        nc.tensor.matmul(
                            ps[:, k - px0:k - px0 + 512],
                            wq, blueb[:, k:k + 512],
                            start=True, stop=(li == 0))
                    if li == 1:
                        for k in range(px0, px0 + 4096, 512):
                            nc.tensor.matmul(
                                ps[:, k - px0:k - px0 + 512],
                                wqg, T2[:, k:k + 512],
                                start=False, stop=True,
                                skip_group_check=True)
                    nc.scalar.activation(
                        T1[:, px0:px0 + 4096], ps[:],
                        ACTF.Identity, bias=qb)

            # prologue: layer-0 convs (drain order K, Q, V: logits-di0
            # needs K rows 0:64 + Q; V only needed at combine)
            emit_kv_conv(0, "k")
            emit_q_conv(0)
            emit_kv_conv(0, "v")

            loop = tc.For_i(0, reps, 1) if reps > 1 else contextlib.nullcontext()
            with loop:
                for li in range(NUM_LAYERS):
                    lwg = vecs[:, li, 3:4]

                    # ---- logits: S[3di:3di+3] = blocksum(Q * shift(K))
                    Qv = T1[:].rearrange("c (h w) -> c h w", w=W)
                    for di in range(3):
                        qb3 = Qv.unsqueeze(1).broadcast_to([C, 3, ROWS, W])
                        kb3 = _restride(
                            Kt[:, 4 * di:4 * di + ROWS, 0:W]
                            .unsqueeze(1).broadcast_to([C, 3, ROWS, W]),
                            1, TS)
                        p3v = big[:].rearrange("c n (h w) -> c n h w", w=W)
                        nc.vector.tensor_mul(p3v, qb3, kb3)
                        v = p3v.rearrange(
                            "c dj (th r) (tw s) -> c dj th tw r s",
                            r=TS, s=TS).rearrange(
                            "c dj th tw r s -> c (dj th) tw r s")
                        nc.vector.reduce_sum(
                            S[:, 3 * di:3 * di + 3, :], v, axis=AX.XY)

                    # ---- softmax over 9 neighbors (logits O(0.3), no max)
                    sf = S[:].rearrange("c n t -> c (n t)")
                    nc.scalar.activation(sf, sf, ACTF.Exp, scale=SCALE)
                    nc.vector.reduce_sum(
                        den[:], S[:].rearrange("c n t -> c t n"), axis=AX.X)
                    nc.vector.reciprocal(den[:], den[:])
                    # divide S by den per-slice so upsample di=0 starts
                    # before the full S is scaled
                    db = den[:].unsqueeze(1)
                    nc.vector.tensor_mul(
                        S[:, 0:3], S[:, 0:3], db.broadcast_to([C, 3, NTOK]))
                    nc.vector.tensor_mul(
                        S[:, 3:9], S[:, 3:9], db.broadcast_to([C, 6, NTOK]))

                    # ---- combine: O = sum_n upsample(A_n) * shift_n(V)
                    O = T2[:]
                    for di in range(3):
                        src = S[:, 3 * di:3 * di + 3, :].rearrange(
                            "c n (th tw) -> c n th tw", tw=NTW).unsqueeze(
                            4).broadcast_to([C, 3, NTH, NTW, TS])
                        nc.vector.tensor_copy(axp[:], src)
                        t3v = big[:].rearrange(
                            "c n (th r x) -> c n th r x", r=TS, x=W)
                        vap = _restride(
                            Vt[:, 4 * di:4 * di + ROWS, 0:W]
                            .unsqueeze(1).broadcast_to([C, 3, ROWS, W]),
                            1, TS).rearrange(
                            "c n (th r) x -> c n th r x", r=TS)
                        aap = axp[:].rearrange(
                            "c n th tw s -> c n th (tw s)").unsqueeze(
                            3).broadcast_to([C, 3, NTH, TS, W])
                        nc.vector.tensor_mul(t3v, vap, aap)
                        if di == 0:
                            nc.vector.tensor_add(O, big[:, 0, :], big[:, 1, :])
                            nc.vector.tensor_add(O, O, big[:, 2, :])
                        else:
                            for n in range(3):
                                nc.vector.tensor_add(O, O, big[:, n, :])

                    # ---- next layer's K conv: PE+ACT work that hides
                    # under this layer's LN DVE stream
                    nli = 1 - li
                    emit_next = (li == 0) or reps > 1
                    if emit_next:
                        emit_kv_conv(nli, "k")

                    # ---- LayerNorm over C via ones-matmul stats
                    o2 = big[:, 0, :]
                    mun = big[:, 1, :]
                    istd = big[:, 2, :]
                    nc.scalar.activation(o2, T2[:], ACTF.Square)
                    for (srcT, dstv, scl) in ((T2[:], mun, -1.0 / C),
                                              (o2, istd, 1.0 / C)):
                        for hx in (0, 4096):
                            for k in range(hx, hx + 4096, 512):
                                nc.tensor.matmul(
                                    ps[:, k - hx:k - hx + 512],
                                    ones[:], srcT[:, k:k + 512],
                                    start=True, stop=True)
                            nc.scalar.activation(
                                dstv[:, hx:hx + 4096], ps[:],
                                ACTF.Identity, scale=scl)
                    # istd holds E[x^2]; mun holds -mu
                    nc.scalar.activation(o2, mun, ACTF.Square)  # mu^2
                    nc.vector.tensor_add(T2[:], T2[:], mun)     # O - mu
                    nc.vector.tensor_sub(istd, istd, o2)        # var
                    nc.scalar.activation(istd, istd, ACTF.Ln, bias=eps_t[:])
                    nc.scalar.activation(istd, istd, ACTF.Exp, scale=-0.5)
                    if emit_next:
                        emit_kv_conv(nli, "v")
                    nc.vector.tensor_mul(T2[:], T2[:], istd)    # N
                    if li == 0:
                        nc.vector.tensor_scalar(
                            acc[:], T2[:], lwg, c0, op0=ALU.mult, op1=ALU.add)
                    else:
                        nc.vector.scalar_tensor_tensor(
                            acc[:], T2[:], lwg, acc[:],
                            op0=ALU.mult, op1=ALU.add)
                    if emit_next:
                        emit_q_conv(nli)

            nc.sync.dma_start(d_out[:], acc[:])

    nc.compile()
    return nc


def _prep_inputs(blue, white, q_w, q_b, k_w, k_b, v_w, v_b, ln_g, ln_b,
                 layer_weights):
    bf16 = ml_dtypes.bfloat16
    f32 = np.float32

    blue = np.asarray(blue, f32)
    whiteP = np.zeros((B, C, H + 2 * TS, W), dtype=f32)
    whiteP[:, :, TS:TS + H, :] = np.asarray(white, f32)

    q_w = np.asarray(q_w, f32)
    q_b = np.asarray(q_b, f32)
    k_w = np.asarray(k_w, f32)
    v_w = np.asarray(v_w, f32)
    ln_b = np.asarray(ln_b, f32)
    ln_g = np.asarray(ln_g, f32)
    lwv = np.asarray(layer_weights, f32)

    wpack = np.zeros((C, NUM_LAYERS, 4 * C), dtype=bf16)
    for li in range(NUM_LAYERS):
        wpack[:, li, 0:C] = q_w[li].T.astype(bf16)
        wpack[:, li, C:2 * C] = k_w[li].T.astype(bf16)
        wpack[:, li, 2 * C:3 * C] = v_w[li].T.astype(bf16)
    # Q1 = Wq1@blue + (Wq1 diag(g0))@N0  (+ qb1 + Wq1@b0)
    wpack[:, 1, 3 * C:4 * C] = (q_w[1].T * ln_g[0][:, None]).astype(bf16)

    vecs = np.zeros((C, NUM_LAYERS, 4), dtype=f32)
    vecs[:, 0, 0] = q_b[0]
    vecs[:, 1, 0] = q_b[1] + q_w[1] @ ln_b[0]
    vecs[:, :, 1] = np.asarray(k_b, f32).T
    vecs[:, :, 2] = np.asarray(v_b, f32).T
    vecs[:, :, 3] = (ln_g * lwv.reshape(NUM_LAYERS, 1)).T

    in_maps = []
    for core in range(8):
        b, half = core // 2, core % 2
        y0 = half * ROWS
        consts = np.zeros((C, 4), f32)
        consts[:, 0] = 0.0 if half == 0 else 1.0
        consts[:, 1] = 0.0 if half == 1 else 1.0
        consts[:, 2] = ln_b[0] * lwv[0] + ln_b[1] * lwv[1]
        in_maps.append({
            "blueb": np.ascontiguousarray(
                blue[b, :, y0:y0 + ROWS, :]).reshape(C, NPIX).astype(bf16),
            "whiteb": np.ascontiguousarray(
                whiteP[b, :, y0:y0 + KROWS, :]).reshape(
                C, KROWS * W).astype(bf16),
            "w": wpack.reshape(C, NUM_LAYERS * 4 * C),
            "vecs": vecs.reshape(C, NUM_LAYERS * 4),
            "consts": consts,
        })
    return in_maps


def kernel(**inputs):
    from concourse.bass_utils import run_bass_kernel_spmd

    reps = int(os.environ.get("KBENCH_REPS", "1"))
    masked = bool(
        np.any(np.asarray(inputs["k_b"])) or np.any(np.asarray(inputs["v_b"])))
    key = ("nc", reps, masked)
    if key not in _CACHE:
        _CACHE[key] = _build(reps, masked)
    nc = _CACHE[key]

    in_maps = _prep_inputs(**inputs)
    res = run_bass_kernel_spmd(nc, in_maps, core_ids=list(range(8)))

    blue = np.asarray(inputs["blue"], np.float32)
    out = np.empty((B, C, H, W), np.float32)
    for core in range(8):
        b, half = core // 2, core % 2
        y0 = half * ROWS
        enh = np.asarray(res.results[core]["out"],
                         np.float32).reshape(C, ROWS, W)
        out[b, :, y0:y0 + ROWS, :] = blue[b, :, y0:y0 + ROWS, :] + enh
    return out


# revision 5
# speedup vs baseline: 34.9913x; 1.4138x over previous
"""Trainium2 Bass kernel for MultiLayerCrossModalAttention (v4).

Contract: kernel(**inputs) takes FULL fp32 inputs, returns FULL [B,C,H,W]
fp32 output. Sharding: core = b*2 + half (batch x H-halves); the white/K/V
side carries a 4-pixel halo so attention needs no cross-core traffic.

v4 design (measured-cost driven):
- All convs in bf16 on PE, batched 8x512 into one [C,4096] PSUM tile
  (~0.25us/matmul), drained by ScalarE Identity(+bias) (~8us/4096).
- Software pipelined: layer li+1's K/V convs are emitted between layer
  li's combine and LayerNorm so PE/ACT conv work hides under DVE streams;
  drain order K,Q,V minimizes the DVE wait at the logits head.
- LayerNorm channel stats via ones-matmul on PE (replaces gpsimd
  partition_all_reduce, ~74us/op -> ~10us).
- current_blue eliminated: Q1 = Wq1@blue + (Wq1 diag(g0))@N0 folded into
  one PSUM accumulation group (host-side weight fold).
- attention combine: 9-term accumulation by wide bf16 adds (2x DVE mode)
  instead of 1x-mode reduces; softmax division folded into S per-slice.
- out = blue + acc is finished on HOST in f32 (device acc is bf16 enh sum).
"""

import os
import sys

import numpy as np

if "/opt/trn_rl_repo" not in sys.path:
    sys.path.insert(0, "/opt/trn_rl_repo")

import ml_dtypes

TS = 4
C = 128
NUM_LAYERS = 2
SCALE = float((TS * TS) ** -0.5)
LN_EPS = 1e-5

B, H, W = 4, 128, 128
ROWS = H // 2
KROWS = ROWS + 2 * TS
PW = W + 2 * TS
NTH = ROWS // TS
NTW = W // TS
NTOK = NTH * NTW
NPIX = ROWS * W

_CACHE = {}


def _restride(ap, dim, step):
    b = ap.copy()
    b.ap[dim] = [step, b.ap[dim][1]]
    return b


def _build(reps=1, masked=False):
    import contextlib
    import concourse.bass as bass
    import concourse.tile as tile
    from concourse import bacc, bass_isa, mybir

    if not getattr(bacc, "_act_tables_patched", False):
        _orig_tables = bacc.get_activation_tables
        _KEEP = "natural_log_exp_and_others"

        def _patched(arch):
            t = _orig_tables(arch)
            mine = t[_KEEP]
            return {
                name: (fns if name == _KEEP else (fns - mine))
                for name, fns in t.items()
            }

        bacc.get_activation_tables = _patched
        bacc._act_tables_patched = True

    F32 = mybir.dt.float32
    BF16 = mybir.dt.bfloat16
    AX = mybir.AxisListType
    ALU = mybir.AluOpType
    ACTF = mybir.ActivationFunctionType

    nc = bacc.Bacc("TRN2", target_bir_lowering=False, debug=False, num_devices=8)

    d_blueb = nc.dram_tensor("blueb", [C, NPIX], BF16, kind="ExternalInput").ap()
    d_whiteb = nc.dram_tensor("whiteb", [C, KROWS * W], BF16,
                              kind="ExternalInput").ap()
    d_w = nc.dram_tensor("w", [C, NUM_LAYERS * 4 * C], BF16,
                         kind="ExternalInput").ap()
    d_vecs = nc.dram_tensor("vecs", [C, NUM_LAYERS * 4], F32,
                            kind="ExternalInput").ap()
    d_consts = nc.dram_tensor("consts", [C, 4], F32, kind="ExternalInput").ap()
    d_out = nc.dram_tensor("out", [C, NPIX], BF16, kind="ExternalOutput").ap()

    with tile.TileContext(nc) as tc:
        with (
            nc.allow_low_precision("bf16 compute by design"),
            tc.tile_pool(name="pp", bufs=1) as pp,
            tc.tile_pool(name="psp", bufs=1, space="PSUM") as psp,
        ):
            acc = pp.tile([C, NPIX], BF16)        # 16K: weighted enh sum
            blueb = pp.tile([C, NPIX], BF16)      # 16K
            whiteb = pp.tile([C, KROWS * W], BF16)  # 18K
            Kt = pp.tile([C, KROWS, PW], BF16)    # 19.1K
            Vt = pp.tile([C, KROWS, PW], BF16)    # 19.1K
            T1 = pp.tile([C, NPIX], BF16)         # 16K: Qt
            T2 = pp.tile([C, NPIX], BF16)         # 16K: O / N (normalized)
            S = pp.tile([C, 9, NTOK], BF16)       # 9K
            den = pp.tile([C, NTOK], F32)         # 2K
            axp = pp.tile([C, 3, NTH, NTW, TS], BF16)  # 12K upsampled attn
            big = pp.tile([C, 3, NPIX], BF16)     # 48K: P3 / tmp3 / LN stats
            wts = pp.tile([C, NUM_LAYERS, 4 * C], BF16, name="wts")  # 2K
            vecs = pp.tile([C, NUM_LAYERS, 4], F32, name="vecs")
            consts = pp.tile([C, 4], F32)
            ones = pp.tile([C, C], BF16)
            eps_t = pp.tile([C, 1], F32)

            nc.sync.dma_start(blueb[:], d_blueb[:])
            nc.sync.dma_start(whiteb[:], d_whiteb[:])
            nc.sync.dma_start(wts[:], d_w[:])
            nc.sync.dma_start(vecs[:], d_vecs[:])
            nc.sync.dma_start(consts[:], d_consts[:])
            nc.vector.memset(eps_t[:], LN_EPS)
            nc.vector.memset(ones[:], 1.0)
            mtop = consts[:, 0:1]
            mbot = consts[:, 1:2]
            c0 = consts[:, 2:3]
            # zero x-margins of Kt/Vt once (drains never write them)
            for t in (Kt, Vt):
                m = _restride(
                    t[:, :, 0:TS].unsqueeze(2).broadcast_to(
                        [C, KROWS, 2, TS]), 2, W + TS)
                nc.gpsimd.memset(m, 0.0)

            ps = psp.tile([C, 4096], F32)

            def emit_conv(wmat, bias, dst, src, npx):
                """1x1 conv src->dst via PE + ACT Identity(+bias) drains."""
                px0 = 0
                while px0 < npx:
                    px1 = min(px0 + 4096, npx)
                    for k in range(px0, px1, 512):
                        nc.tensor.matmul(
                            ps[:, k - px0:k - px0 + 512],
                            wmat, src[:, k:k + 512], start=True, stop=True)
                    if dst is None:
                        o = T1[:, px0:px1]
                        i = ps[:, 0:px1 - px0]
                    else:
                        o = dst[:, px0 // W:px1 // W, TS:TS + W]
                        i = ps[:, 0:px1 - px0].rearrange("c (h w) -> c h w", w=W)
                    nc.scalar.activation(o, i, ACTF.Identity, bias=bias)
                    px0 = px1

            def emit_kv_conv(li, which):
                if which == "k":
                    wmat, bias, dst = wts[:, li, C:2 * C], vecs[:, li, 1:2], Kt
                else:
                    wmat, bias, dst = wts[:, li, 2 * C:3 * C], vecs[:, li, 2:3], Vt
                emit_conv(wmat, bias, dst, whiteb[:], KROWS * W)
                if masked:
                    nc.vector.tensor_scalar_mul(
                        dst[:, 0:TS, :], dst[:, 0:TS, :], mtop)
                    nc.vector.tensor_scalar_mul(
                        dst[:, ROWS + TS:KROWS, :],
                        dst[:, ROWS + TS:KROWS, :], mbot)

            def emit_q_conv(li):
                # li0: wq@blue; li1: wq@blue + wqg@N0 (PSUM accumulation)
                wq = wts[:, li, 0:C]
                wqg = wts[:, li, 3 * C:4 * C]
                qb = vecs[:, li, 0:1]
                for px0 in (0, 4096):
                    for k in range(px0, px0 + 4096, 512):
                        nc.tensor.matmul(
                            ps[:, k - px0:k - px0 + 512],
                            wq, blueb[:, k:k + 512],
                            start=True, stop=(li == 0))
                    if li == 1:
                        for k in range(px0, px0 + 4096, 512):
                            nc.tensor.matmul(
                                ps[:, k - px0:k - px0 + 512],
                                wqg, T2[:, k:k + 512],
                                start=False, stop=True,
                                skip_group_check=True)
                    nc.scalar.activation(
                        T1[:, px0:px0 + 4096], ps[:],
                        ACTF.Identity, bias=qb)

            # prologue: layer-0 convs (drain order K, Q, V: logits-di0
            # needs K rows 0:64 + Q; V only needed at combine)
            emit_kv_conv(0, "k")
            emit_q_conv(0)
            emit_kv_conv(0, "v")

            loop = tc.For_i(0, reps, 1) if reps > 1 else contextlib.nullcontext()
            with loop:
                for li in range(NUM_LAYERS):
                    lwg = vecs[:, li, 3:4]

                    # ---- logits: S[3di:3di+3] = blocksum(Q * shift(K))
                    Qv = T1[:].rearrange("c (h w) -> c h w", w=W)
                    for di in range(3):
                        qb3 = Qv.unsqueeze(1).broadcast_to([C, 3, ROWS, W])
                        kb3 = _restride(
                            Kt[:, 4 * di:4 * di + ROWS, 0:W]
                            .unsqueeze(1).broadcast_to([C, 3, ROWS, W]),
                            1, TS)
                        p3v = big[:].rearrange("c n (h w) -> c n h w", w=W)
                        nc.vector.tensor_mul(p3v, qb3, kb3)
                        v = p3v.rearrange(
                            "c dj (th r) (tw s) -> c dj th tw r s",
                            r=TS, s=TS).rearrange(
                            "c dj th tw r s -> c (dj th) tw r s")
                        nc.vector.reduce_sum(
                            S[:, 3 * di:3 * di + 3, :], v, axis=AX.XY)

                    # ---- softmax over 9 neighbors (logits O(0.3), no max)
                    sf = S[:].rearrange("c n t -> c (n t)")
                    nc.scalar.activation(sf, sf, ACTF.Exp, scale=SCALE)
                    nc.vector.reduce_sum(
                        den[:], S[:].rearrange("c n t -> c t n"), axis=AX.X)
                    nc.vector.reciprocal(den[:], den[:])
                    # divide S by den per-slice so upsample di=0 starts
                    # before the full S is scaled
                    db = den[:].unsqueeze(1)
                    nc.vector.tensor_mul(
                        S[:, 0:3], S[:, 0:3], db.broadcast_to([C, 3, NTOK]))
                    nc.vector.tensor_mul(
                        S[:, 3:9], S[:, 3:9], db.broadcast_to([C, 6, NTOK]))

                    # ---- combine: O = sum_n upsample(A_n) * shift_n(V)
                    O = T2[:]
                    for di in range(3):
                        src = S[:, 3 * di:3 * di + 3, :].rearrange(
                            "c n (th tw) -> c n th tw", tw=NTW).unsqueeze(
                            4).broadcast_to([C, 3, NTH, NTW, TS])
                        nc.vector.tensor_copy(axp[:], src)
                        t3v = big[:].rearrange(
                            "c n (th r x) -> c n th r x", r=TS, x=W)
                        vap = _restride(
                            Vt[:, 4 * di:4 * di + ROWS, 0:W]
                            .unsqueeze(1).broadcast_to([C, 3, ROWS, W]),
                            1, TS).rearrange(
                            "c n (th r) x -> c n th r x", r=TS)
                        aap = axp[:].rearrange(
                            "c n th tw s -> c n th (tw s)").unsqueeze(
                            3).broadcast_to([C, 3, NTH, TS, W])
                        nc.vector.tensor_mul(t3v, vap, aap)
                        if di == 0:
                            nc.vector.tensor_add(O, big[:, 0, :], big[:, 1, :])
                            nc.vector.tensor_add(O, O, big[:, 2, :])
                        else:
                            for n in range(3):
                                nc.vector.tensor_add(O, O, big[:, n, :])

                    # ---- next layer's K conv: PE+ACT work that hides
                    # under this layer's LN DVE stream
                    nli = 1 - li
                    emit_next = (li == 0) or reps > 1
                    if emit_next:
                        emit_kv_conv(nli, "k")

                    # ---- LayerNorm over C via ones-matmul stats
                    o2 = big[:, 0, :]
                    mun = big[:, 1, :]
                    istd = big[:, 2, :]
                    nc.scalar.activation(o2, T2[:], ACTF.Square)
                    for (srcT, dstv, scl) in ((T2[:], mun, -1.0 / C),
                                              (o2, istd, 1.0 / C)):
                        for hx in (0, 4096):
                            for k in range(hx, hx + 4096, 512):
                                nc.tensor.matmul(
                                    ps[:, k - hx:k - hx + 512],
                                    ones[:], srcT[:, k:k + 512],
                                    start=True, stop=True)
                            nc.scalar.activation(
                                dstv[:, hx:hx + 4096], ps[:],
                                ACTF.Identity, scale=scl)
                    # istd holds E[x^2]; mun holds -mu
                    nc.scalar.activation(o2, mun, ACTF.Square)  # mu^2
                    nc.vector.tensor_add(T2[:], T2[:], mun)     # O - mu
                    nc.vector.tensor_sub(istd, istd, o2)        # var
                    nc.scalar.activation(istd, istd, ACTF.Ln, bias=eps_t[:])
                    nc.scalar.activation(istd, istd, ACTF.Exp, scale=-0.5)
                    if emit_next:
                        emit_kv_conv(nli, "v")
                    nc.vector.tensor_mul(T2[:], T2[:], istd)    # N
                    if li == 0:
                        nc.vector.tensor_scalar(
                            acc[:], T2[:], lwg, c0, op0=ALU.mult, op1=ALU.add)
                    else:
                        nc.vector.scalar_tensor_tensor(
                            acc[:], T2[:], lwg, acc[:],
                            op0=ALU.mult, op1=ALU.add)
                    if emit_next:
                        emit_q_conv(nli)

            nc.sync.dma_start(d_out[:], acc[:])

    nc.compile()
    return nc


def _prep_inputs(blue, white, q_w, q_b, k_w, k_b, v_w, v_b, ln_g, ln_b,
                 layer_weights):
    bf16 = ml_dtypes.bfloat16
    f32 = np.float32

    blue = np.asarray(blue, f32)
    whiteP = np.zeros((B, C, H + 2 * TS, W), dtype=f32)
    whiteP[:, :, TS:TS + H, :] = np.asarray(white, f32)

    q_w = np.asarray(q_w, f32)
    q_b = np.asarray(q_b, f32)
    k_w = np.asarray(k_w, f32)
    v_w = np.asarray(v_w, f32)
    ln_b = np.asarray(ln_b, f32)
    ln_g = np.asarray(ln_g, f32)
    lwv = np.asarray(layer_weights, f32)

    wpack = np.zeros((C, NUM_LAYERS, 4 * C), dtype=bf16)
    for li in range(NUM_LAYERS):
        wpack[:, li, 0:C] = q_w[li].T.astype(bf16)
        wpack[:, li, C:2 * C] = k_w[li].T.astype(bf16)
        wpack[:, li, 2 * C:3 * C] = v_w[li].T.astype(bf16)
    # Q1 = Wq1@blue + (Wq1 diag(g0))@N0  (+ qb1 + Wq1@b0)
    wpack[:, 1, 3 * C:4 * C] = (q_w[1].T * ln_g[0][:, None]).astype(bf16)

    vecs = np.zeros((C, NUM_LAYERS, 4), dtype=f32)
    vecs[:, 0, 0] = q_b[0]
    vecs[:, 1, 0] = q_b[1] + q_w[1] @ ln_b[0]
    vecs[:, :, 1] = np.asarray(k_b, f32).T
    vecs[:, :, 2] = np.asarray(v_b, f32).T
    vecs[:, :, 3] = (ln_g * lwv.reshape(NUM_LAYERS, 1)).T

    in_maps = []
    for core in range(8):
        b, half = core // 2, core % 2
        y0 = half * ROWS
        consts = np.zeros((C, 4), f32)
        consts[:, 0] = 0.0 if half == 0 else 1.0
        consts[:, 1] = 0.0 if half == 1 else 1.0
        consts[:, 2] = ln_b[0] * lwv[0] + ln_b[1] * lwv[1]
        in_maps.append({
            "blueb": np.ascontiguousarray(
                blue[b, :, y0:y0 + ROWS, :]).reshape(C, NPIX).astype(bf16),
            "whiteb": np.ascontiguousarray(
                whiteP[b, :, y0:y0 + KROWS, :]).reshape(
                C, KROWS * W).astype(bf16),
            "w": wpack.reshape(C, NUM_LAYERS * 4 * C),
            "vecs": vecs.reshape(C, NUM_LAYERS * 4),
            "consts": consts,
        })
    return in_maps


def kernel(**inputs):
    from concourse.bass_utils import run_bass_kernel_spmd

    reps = int(os.environ.get("KBENCH_REPS", "1"))
    masked = bool(
        np.any(np.asarray(inputs["k_b"])) or np.any(np.asarray(inputs["v_b"])))
    key = ("nc", reps, masked)
    if key not in _CACHE:
        _CACHE[key] = _build(reps, masked)
    nc = _CACHE[key]

    in_maps = _prep_inputs(**inputs)
    res = run_bass_kernel_spmd(nc, in_maps, core_ids=list(range(8)))

    blue = np.asarray(inputs["blue"], np.float32)
    out = np.empty((B, C, H, W), np.float32)
    for core in range(8):
        b, half = core // 2, core % 2
        y0 = half * ROWS
        enh = np.asarray(res.results[core]["out"],
                         np.float32).reshape(C, ROWS, W)
        out[b, :, y0:y0 + ROWS, :] = blue[b, :, y0:y0 + ROWS, :] + enh
    return out


# revision 6
# speedup vs baseline: 36.1547x; 1.0332x over previous
"""Trainium2 Bass kernel for MultiLayerCrossModalAttention (v4).

Contract: kernel(**inputs) takes FULL fp32 inputs, returns FULL [B,C,H,W]
fp32 output. Sharding: core = b*2 + half (batch x H-halves); the white/K/V
side carries a 4-pixel halo so attention needs no cross-core traffic.

v4 design (measured-cost driven):
- All convs in bf16 on PE, batched 8x512 into one [C,4096] PSUM tile
  (~0.25us/matmul), drained by ScalarE Identity(+bias) (~8us/4096).
- Software pipelined: layer li+1's K/V convs are emitted between layer
  li's combine and LayerNorm so PE/ACT conv work hides under DVE streams;
  drain order K,Q,V minimizes the DVE wait at the logits head.
- LayerNorm channel stats via ones-matmul on PE (replaces gpsimd
  partition_all_reduce, ~74us/op -> ~10us).
- current_blue eliminated: Q1 = Wq1@blue + (Wq1 diag(g0))@N0 folded into
  one PSUM accumulation group (host-side weight fold).
- attention combine: 9-term accumulation by wide bf16 adds (2x DVE mode)
  instead of 1x-mode reduces; softmax division folded into S per-slice.
- out = blue + acc is finished on HOST in f32 (device acc is bf16 enh sum).
"""

import os
import sys

import numpy as np

if "/opt/trn_rl_repo" not in sys.path:
    sys.path.insert(0, "/opt/trn_rl_repo")

import ml_dtypes

TS = 4
C = 128
NUM_LAYERS = 2
SCALE = float((TS * TS) ** -0.5)
LN_EPS = 1e-5

B, H, W = 4, 128, 128
ROWS = H // 2
KROWS = ROWS + 2 * TS
PW = W + 2 * TS
NTH = ROWS // TS
NTW = W // TS
NTOK = NTH * NTW
NPIX = ROWS * W

_CACHE = {}


def _restride(ap, dim, step):
    b = ap.copy()
    b.ap[dim] = [step, b.ap[dim][1]]
    return b


def _build(reps=1, masked=False):
    import contextlib
    import concourse.bass as bass
    import concourse.tile as tile
    from concourse import bacc, bass_isa, mybir

    if not getattr(bacc, "_act_tables_patched", False):
        _orig_tables = bacc.get_activation_tables
        _KEEP = "natural_log_exp_and_others"

        def _patched(arch):
            t = _orig_tables(arch)
            mine = t[_KEEP]
            return {
                name: (fns if name == _KEEP else (fns - mine))
                for name, fns in t.items()
            }

        bacc.get_activation_tables = _patched
        bacc._act_tables_patched = True

    F32 = mybir.dt.float32
    BF16 = mybir.dt.bfloat16
    AX = mybir.AxisListType
    ALU = mybir.AluOpType
    ACTF = mybir.ActivationFunctionType

    nc = bacc.Bacc("TRN2", target_bir_lowering=False, debug=False, num_devices=8)

    d_blueb = nc.dram_tensor("blueb", [C, NPIX], BF16, kind="ExternalInput").ap()
    d_whiteb = nc.dram_tensor("whiteb", [C, KROWS * W], BF16,
                              kind="ExternalInput").ap()
    d_w = nc.dram_tensor("w", [C, NUM_LAYERS * 4 * C], BF16,
                         kind="ExternalInput").ap()
    d_vecs = nc.dram_tensor("vecs", [C, NUM_LAYERS * 4], F32,
                            kind="ExternalInput").ap()
    d_consts = nc.dram_tensor("consts", [C, 4], F32, kind="ExternalInput").ap()
    d_out = nc.dram_tensor("out", [C, NPIX], BF16, kind="ExternalOutput").ap()

    with tile.TileContext(nc) as tc:
        with (
            nc.allow_low_precision("bf16 compute by design"),
            tc.tile_pool(name="pp", bufs=1) as pp,
            tc.tile_pool(name="psp", bufs=1, space="PSUM") as psp,
        ):
            acc = pp.tile([C, NPIX], BF16)        # 16K: weighted enh sum
            blueb = pp.tile([C, NPIX], BF16)      # 16K
            whiteb = pp.tile([C, KROWS * W], BF16)  # 18K
            Kt = pp.tile([C, KROWS, PW], BF16)    # 19.1K
            Vt = pp.tile([C, KROWS, PW], BF16)    # 19.1K
            T1 = pp.tile([C, NPIX], BF16)         # 16K: Qt
            T2 = pp.tile([C, NPIX], BF16)         # 16K: O / N (normalized)
            S = pp.tile([C, 9, NTOK], BF16)       # 9K
            den = pp.tile([C, NTOK], F32)         # 2K
            axp = pp.tile([C, 3, NTH, NTW, TS], BF16)  # 12K upsampled attn
            big = pp.tile([C, 3, NPIX], BF16)     # 48K: P3 / tmp3 / LN stats
            wts = pp.tile([C, NUM_LAYERS, 4 * C], BF16, name="wts")  # 2K
            vecs = pp.tile([C, NUM_LAYERS, 4], F32, name="vecs")
            consts = pp.tile([C, 4], F32)
            ones = pp.tile([C, C], BF16)
            eps_t = pp.tile([C, 1], F32)

            nc.sync.dma_start(blueb[:], d_blueb[:])
            nc.sync.dma_start(whiteb[:], d_whiteb[:])
            nc.sync.dma_start(wts[:], d_w[:])
            nc.sync.dma_start(vecs[:], d_vecs[:])
            nc.sync.dma_start(consts[:], d_consts[:])
            nc.vector.memset(eps_t[:], LN_EPS)
            nc.vector.memset(ones[:], 1.0)
            mtop = consts[:, 0:1]
            mbot = consts[:, 1:2]
            c0 = consts[:, 2:3]
            # zero x-margins of Kt/Vt once (drains never write them)
            for t in (Kt, Vt):
                m = _restride(
                    t[:, :, 0:TS].unsqueeze(2).broadcast_to(
                        [C, KROWS, 2, TS]), 2, W + TS)
                nc.gpsimd.memset(m, 0.0)

            ps = psp.tile([C, 4096], F32)

            def emit_conv(wmat, bias, dst, src, npx):
                """1x1 conv src->dst via PE + ACT Identity(+bias) drains."""
                px0 = 0
                while px0 < npx:
                    px1 = min(px0 + 4096, npx)
                    for k in range(px0, px1, 512):
                        nc.tensor.matmul(
                            ps[:, k - px0:k - px0 + 512],
                            wmat, src[:, k:k + 512], start=True, stop=True)
                    if dst is None:
                        o = T1[:, px0:px1]
                        i = ps[:, 0:px1 - px0]
                    else:
                        o = dst[:, px0 // W:px1 // W, TS:TS + W]
                        i = ps[:, 0:px1 - px0].rearrange("c (h w) -> c h w", w=W)
                    nc.scalar.activation(o, i, ACTF.Identity, bias=bias)
                    px0 = px1

            def emit_kv_conv(li, which):
                if which == "k":
                    wmat, bias, dst = wts[:, li, C:2 * C], vecs[:, li, 1:2], Kt
                else:
                    wmat, bias, dst = wts[:, li, 2 * C:3 * C], vecs[:, li, 2:3], Vt
                emit_conv(wmat, bias, dst, whiteb[:], KROWS * W)
                if masked:
                    nc.vector.tensor_scalar_mul(
                        dst[:, 0:TS, :], dst[:, 0:TS, :], mtop)
                    nc.vector.tensor_scalar_mul(
                        dst[:, ROWS + TS:KROWS, :],
                        dst[:, ROWS + TS:KROWS, :], mbot)

            def emit_q_conv(li):
                # li0: wq@blue; li1: wq@blue + wqg@N0 (PSUM accumulation)
                wq = wts[:, li, 0:C]
                wqg = wts[:, li, 3 * C:4 * C]
                qb = vecs[:, li, 0:1]
                for px0 in (0, 4096):
                    for k in range(px0, px0 + 4096, 512):
                        nc.tensor.matmul(
                            ps[:, k - px0:k - px0 + 512],
                            wq, blueb[:, k:k + 512],
                            start=True, stop=(li == 0))
                    if li == 1:
                        for k in range(px0, px0 + 4096, 512):
                            nc.tensor.matmul(
                                ps[:, k - px0:k - px0 + 512],
                                wqg, T2[:, k:k + 512],
                                start=False, stop=True,
                                skip_group_check=True)
                    nc.scalar.activation(
                        T1[:, px0:px0 + 4096], ps[:],
                        ACTF.Identity, bias=qb)

            # prologue: layer-0 convs (drain order K, Q, V: logits-di0
            # needs K rows 0:64 + Q; V only needed at combine)
            emit_kv_conv(0, "k")
            emit_q_conv(0)
            emit_kv_conv(0, "v")

            loop = tc.For_i(0, reps, 1) if reps > 1 else contextlib.nullcontext()
            with loop:
                for li in range(NUM_LAYERS):
                    lwg = vecs[:, li, 3:4]

                    # ---- logits: S[3di:3di+3] = blocksum(Q * shift(K))
                    Qv = T1[:].rearrange("c (h w) -> c h w", w=W)
                    for di in range(3):
                        qb3 = Qv.unsqueeze(1).broadcast_to([C, 3, ROWS, W])
                        kb3 = _restride(
                            Kt[:, 4 * di:4 * di + ROWS, 0:W]
                            .unsqueeze(1).broadcast_to([C, 3, ROWS, W]),
                            1, TS)
                        p3v = big[:].rearrange("c n (h w) -> c n h w", w=W)
                        nc.vector.tensor_mul(p3v, qb3, kb3)
                        v = p3v.rearrange(
                            "c dj (th r) (tw s) -> c dj th tw r s",
                            r=TS, s=TS).rearrange(
                            "c dj th tw r s -> c (dj th) tw r s")
                        nc.vector.reduce_sum(
                            S[:, 3 * di:3 * di + 3, :], v, axis=AX.XY)

                    # ---- softmax over 9 neighbors (logits O(0.3), no max)
                    sf = S[:].rearrange("c n t -> c (n t)")
                    nc.scalar.activation(sf, sf, ACTF.Exp, scale=SCALE)
                    nc.vector.reduce_sum(
                        den[:], S[:].rearrange("c n t -> c t n"), axis=AX.X)
                    nc.vector.reciprocal(den[:], den[:])
                    # divide S by den per-slice so upsample di=0 starts
                    # before the full S is scaled
                    db = den[:].unsqueeze(1)
                    nc.vector.tensor_mul(
                        S[:, 0:3], S[:, 0:3], db.broadcast_to([C, 3, NTOK]))
                    nc.vector.tensor_mul(
                        S[:, 3:9], S[:, 3:9], db.broadcast_to([C, 6, NTOK]))

                    # ---- combine: O = sum_n upsample(A_n) * shift_n(V)
                    O = T2[:]
                    for di in range(3):
                        src = S[:, 3 * di:3 * di + 3, :].rearrange(
                            "c n (th tw) -> c n th tw", tw=NTW).unsqueeze(
                            4).broadcast_to([C, 3, NTH, NTW, TS])
                        nc.vector.tensor_copy(axp[:], src)
                        t3v = big[:].rearrange(
                            "c n (th r x) -> c n th r x", r=TS, x=W)
                        vap = _restride(
                            Vt[:, 4 * di:4 * di + ROWS, 0:W]
                            .unsqueeze(1).broadcast_to([C, 3, ROWS, W]),
                            1, TS).rearrange(
                            "c n (th r) x -> c n th r x", r=TS)
                        aap = axp[:].rearrange(
                            "c n th tw s -> c n th (tw s)").unsqueeze(
                            3).broadcast_to([C, 3, NTH, TS, W])
                        nc.vector.tensor_mul(t3v, vap, aap)
                        if di == 0:
                            nc.vector.tensor_add(O, big[:, 0, :], big[:, 1, :])
                            nc.vector.tensor_add(O, O, big[:, 2, :])
                        else:
                            for n in range(3):
                                nc.vector.tensor_add(O, O, big[:, n, :])

                    # ---- next layer's K conv: PE+ACT work that hides
                    # under this layer's LN DVE stream
                    nli = 1 - li
                    emit_next = (li == 0) or reps > 1
                    if emit_next:
                        emit_kv_conv(nli, "k")
                        emit_kv_conv(nli, "v")

                    # ---- LayerNorm over C via ones-matmul stats
                    o2 = big[:, 0, :]
                    mun = big[:, 1, :]
                    istd = big[:, 2, :]
                    nc.scalar.activation(o2, T2[:], ACTF.Square)
                    for (srcT, dstv, scl) in ((T2[:], mun, -1.0 / C),
                                              (o2, istd, 1.0 / C)):
                        for hx in (0, 4096):
                            for k in range(hx, hx + 4096, 512):
                                nc.tensor.matmul(
                                    ps[:, k - hx:k - hx + 512],
                                    ones[:], srcT[:, k:k + 512],
                                    start=True, stop=True)
                            nc.scalar.activation(
                                dstv[:, hx:hx + 4096], ps[:],
                                ACTF.Identity, scale=scl)
                    # istd holds E[x^2]; mun holds -mu
                    nc.scalar.activation(o2, mun, ACTF.Square)  # mu^2
                    nc.vector.tensor_add(T2[:], T2[:], mun)     # O - mu
                    nc.vector.tensor_sub(istd, istd, o2)        # var
                    nc.scalar.activation(istd, istd, ACTF.Ln, bias=eps_t[:])
                    nc.scalar.activation(istd, istd, ACTF.Exp, scale=-0.5)
                    nc.vector.tensor_mul(T2[:], T2[:], istd)    # N
                    if li == 0:
                        nc.vector.tensor_scalar(
                            acc[:], T2[:], lwg, c0, op0=ALU.mult, op1=ALU.add)
                    else:
                        nc.vector.scalar_tensor_tensor(
                            acc[:], T2[:], lwg, acc[:],
                            op0=ALU.mult, op1=ALU.add)
                    if emit_next:
                        emit_q_conv(nli)

            nc.sync.dma_start(d_out[:], acc[:])

    nc.compile()
    return nc


def _prep_inputs(blue, white, q_w, q_b, k_w, k_b, v_w, v_b, ln_g, ln_b,
                 layer_weights):
    bf16 = ml_dtypes.bfloat16
    f32 = np.float32

    blue = np.asarray(blue, f32)
    whiteP = np.zeros((B, C, H + 2 * TS, W), dtype=f32)
    whiteP[:, :, TS:TS + H, :] = np.asarray(white, f32)

    q_w = np.asarray(q_w, f32)
    q_b = np.asarray(q_b, f32)
    k_w = np.asarray(k_w, f32)
    v_w = np.asarray(v_w, f32)
    ln_b = np.asarray(ln_b, f32)
    ln_g = np.asarray(ln_g, f32)
    lwv = np.asarray(layer_weights, f32)

    wpack = np.zeros((C, NUM_LAYERS, 4 * C), dtype=bf16)
    for li in range(NUM_LAYERS):
        wpack[:, li, 0:C] = q_w[li].T.astype(bf16)
        wpack[:, li, C:2 * C] = k_w[li].T.astype(bf16)
        wpack[:, li, 2 * C:3 * C] = v_w[li].T.astype(bf16)
    # Q1 = Wq1@blue + (Wq1 diag(g0))@N0  (+ qb1 + Wq1@b0)
    wpack[:, 1, 3 * C:4 * C] = (q_w[1].T * ln_g[0][:, None]).astype(bf16)

    vecs = np.zeros((C, NUM_LAYERS, 4), dtype=f32)
    vecs[:, 0, 0] = q_b[0]
    vecs[:, 1, 0] = q_b[1] + q_w[1] @ ln_b[0]
    vecs[:, :, 1] = np.asarray(k_b, f32).T
    vecs[:, :, 2] = np.asarray(v_b, f32).T
    vecs[:, :, 3] = (ln_g * lwv.reshape(NUM_LAYERS, 1)).T

    in_maps = []
    for core in range(8):
        b, half = core // 2, core % 2
        y0 = half * ROWS
        consts = np.zeros((C, 4), f32)
        consts[:, 0] = 0.0 if half == 0 else 1.0
        consts[:, 1] = 0.0 if half == 1 else 1.0
        consts[:, 2] = ln_b[0] * lwv[0] + ln_b[1] * lwv[1]
        in_maps.append({
            "blueb": np.ascontiguousarray(
                blue[b, :, y0:y0 + ROWS, :]).reshape(C, NPIX).astype(bf16),
            "whiteb": np.ascontiguousarray(
                whiteP[b, :, y0:y0 + KROWS, :]).reshape(
                C, KROWS * W).astype(bf16),
            "w": wpack.reshape(C, NUM_LAYERS * 4 * C),
            "vecs": vecs.reshape(C, NUM_LAYERS * 4),
            "consts": consts,
        })
    return in_maps


def kernel(**inputs):
    from concourse.bass_utils import run_bass_kernel_spmd

    reps = int(os.environ.get("KBENCH_REPS", "1"))
    masked = bool(
        np.any(np.asarray(inputs["k_b"])) or np.any(np.asarray(inputs["v_b"])))
    key = ("nc", reps, masked)
    if key not in _CACHE:
        _CACHE[key] = _build(reps, masked)
    nc = _CACHE[key]

    in_maps = _prep_inputs(**inputs)
    res = run_bass_kernel_spmd(nc, in_maps, core_ids=list(range(8)))

    blue = np.asarray(inputs["blue"], np.float32)
    out = np.empty((B, C, H, W), np.float32)
    for core in range(8):
        b, half = core // 2, core % 2
        y0 = half * ROWS
        enh = np.asarray(res.results[core]["out"],
                         np.float32).reshape(C, ROWS, W)
        out[b, :, y0:y0 + ROWS, :] = blue[b, :, y0:y0 + ROWS, :] + enh
    return out


# revision 7
# speedup vs baseline: 36.9932x; 1.0232x over previous
"""Trainium2 Bass kernel for MultiLayerCrossModalAttention (v4).

Contract: kernel(**inputs) takes FULL fp32 inputs, returns FULL [B,C,H,W]
fp32 output. Sharding: core = b*2 + half (batch x H-halves); the white/K/V
side carries a 4-pixel halo so attention needs no cross-core traffic.

v4 design (measured-cost driven):
- All convs in bf16 on PE, batched 8x512 into one [C,4096] PSUM tile
  (~0.25us/matmul), drained by ScalarE Identity(+bias) (~8us/4096).
- Software pipelined: layer li+1's K and V convs are both emitted between
  layer li's combine and LayerNorm (A/B-measured best order) so PE/ACT
  conv work hides under DVE streams; Q follows the LN apply it depends on.
- LayerNorm channel stats via ones-matmul on PE (replaces gpsimd
  partition_all_reduce, ~74us/op -> ~10us).
- current_blue eliminated: Q1 = Wq1@blue + (Wq1 diag(g0))@N0 folded into
  one PSUM accumulation group (host-side weight fold).
- attention combine: 9-term accumulation by wide bf16 adds (2x DVE mode)
  instead of 1x-mode reduces; softmax division folded into S per-slice.
- out = blue + acc is finished on HOST in f32 (device acc is bf16 enh sum).
"""

import os
import sys

import numpy as np

if "/opt/trn_rl_repo" not in sys.path:
    sys.path.insert(0, "/opt/trn_rl_repo")

import ml_dtypes

TS = 4
C = 128
NUM_LAYERS = 2
SCALE = float((TS * TS) ** -0.5)
LN_EPS = 1e-5

B, H, W = 4, 128, 128
ROWS = H // 2
KROWS = ROWS + 2 * TS
PW = W + 2 * TS
NTH = ROWS // TS
NTW = W // TS
NTOK = NTH * NTW
NPIX = ROWS * W

_CACHE = {}


def _restride(ap, dim, step):
    b = ap.copy()
    b.ap[dim] = [step, b.ap[dim][1]]
    return b


def _build(reps=1, masked=False):
    import contextlib
    import concourse.bass as bass
    import concourse.tile as tile
    from concourse import bacc, bass_isa, mybir

    if not getattr(bacc, "_act_tables_patched", False):
        _orig_tables = bacc.get_activation_tables
        _KEEP = "natural_log_exp_and_others"

        def _patched(arch):
            t = _orig_tables(arch)
            mine = t[_KEEP]
            return {
                name: (fns if name == _KEEP else (fns - mine))
                for name, fns in t.items()
            }

        bacc.get_activation_tables = _patched
        bacc._act_tables_patched = True

    F32 = mybir.dt.float32
    BF16 = mybir.dt.bfloat16
    AX = mybir.AxisListType
    ALU = mybir.AluOpType
    ACTF = mybir.ActivationFunctionType

    nc = bacc.Bacc("TRN2", target_bir_lowering=False, debug=False, num_devices=8)

    d_blueb = nc.dram_tensor("blueb", [C, NPIX], BF16, kind="ExternalInput").ap()
    d_whiteb = nc.dram_tensor("whiteb", [C, KROWS * W], BF16,
                              kind="ExternalInput").ap()
    d_w = nc.dram_tensor("w", [C, NUM_LAYERS * 4 * C], BF16,
                         kind="ExternalInput").ap()
    d_vecs = nc.dram_tensor("vecs", [C, NUM_LAYERS * 4], F32,
                            kind="ExternalInput").ap()
    d_consts = nc.dram_tensor("consts", [C, 4], F32, kind="ExternalInput").ap()
    d_out = nc.dram_tensor("out", [C, NPIX], BF16, kind="ExternalOutput").ap()

    with tile.TileContext(nc) as tc:
        with (
            nc.allow_low_precision("bf16 compute by design"),
            tc.tile_pool(name="pp", bufs=1) as pp,
            tc.tile_pool(name="psp", bufs=1, space="PSUM") as psp,
        ):
            acc = pp.tile([C, NPIX], BF16)        # 16K: weighted enh sum
            blueb = pp.tile([C, NPIX], BF16)      # 16K
            whiteb = pp.tile([C, KROWS * W], BF16)  # 18K
            Kt = pp.tile([C, KROWS, PW], BF16)    # 19.1K
            Vt = pp.tile([C, KROWS, PW], BF16)    # 19.1K
            T1 = pp.tile([C, NPIX], BF16)         # 16K: Qt
            T2 = pp.tile([C, NPIX], BF16)         # 16K: O / N (normalized)
            S = pp.tile([C, 9, NTOK], BF16)       # 9K
            den = pp.tile([C, NTOK], F32)         # 2K
            axp = pp.tile([C, 3, NTH, NTW, TS], BF16)  # 12K upsampled attn
            big = pp.tile([C, 3, NPIX], BF16)     # 48K: P3 / tmp3 / LN stats
            wts = pp.tile([C, NUM_LAYERS, 4 * C], BF16, name="wts")  # 2K
            vecs = pp.tile([C, NUM_LAYERS, 4], F32, name="vecs")
            consts = pp.tile([C, 4], F32)
            ones = pp.tile([C, C], BF16)
            eps_t = pp.tile([C, 1], F32)

            nc.sync.dma_start(blueb[:], d_blueb[:])
            nc.sync.dma_start(whiteb[:], d_whiteb[:])
            nc.sync.dma_start(wts[:], d_w[:])
            nc.sync.dma_start(vecs[:], d_vecs[:])
            nc.sync.dma_start(consts[:], d_consts[:])
            nc.vector.memset(eps_t[:], LN_EPS)
            nc.vector.memset(ones[:], 1.0)
            mtop = consts[:, 0:1]
            mbot = consts[:, 1:2]
            c0 = consts[:, 2:3]
            # zero x-margins of Kt/Vt once (drains never write them)
            for t in (Kt, Vt):
                m = _restride(
                    t[:, :, 0:TS].unsqueeze(2).broadcast_to(
                        [C, KROWS, 2, TS]), 2, W + TS)
                nc.gpsimd.memset(m, 0.0)

            ps = psp.tile([C, 4096], F32)

            def emit_conv(wmat, bias, dst, src, npx):
                """1x1 conv src->dst via PE + ACT Identity(+bias) drains."""
                px0 = 0
                while px0 < npx:
                    px1 = min(px0 + 4096, npx)
                    for k in range(px0, px1, 512):
                        nc.tensor.matmul(
                            ps[:, k - px0:k - px0 + 512],
                            wmat, src[:, k:k + 512], start=True, stop=True)
                    if dst is None:
                        o = T1[:, px0:px1]
                        i = ps[:, 0:px1 - px0]
                    else:
                        o = dst[:, px0 // W:px1 // W, TS:TS + W]
                        i = ps[:, 0:px1 - px0].rearrange("c (h w) -> c h w", w=W)
                    nc.scalar.activation(o, i, ACTF.Identity, bias=bias)
                    px0 = px1

            def emit_kv_conv(li, which):
                if which == "k":
                    wmat, bias, dst = wts[:, li, C:2 * C], vecs[:, li, 1:2], Kt
                else:
                    wmat, bias, dst = wts[:, li, 2 * C:3 * C], vecs[:, li, 2:3], Vt
                emit_conv(wmat, bias, dst, whiteb[:], KROWS * W)
                if masked:
                    nc.vector.tensor_scalar_mul(
                        dst[:, 0:TS, :], dst[:, 0:TS, :], mtop)
                    nc.vector.tensor_scalar_mul(
                        dst[:, ROWS + TS:KROWS, :],
                        dst[:, ROWS + TS:KROWS, :], mbot)

            def emit_q_conv(li):
                # li0: wq@blue; li1: wq@blue + wqg@N0 (PSUM accumulation)
                wq = wts[:, li, 0:C]
                wqg = wts[:, li, 3 * C:4 * C]
                qb = vecs[:, li, 0:1]
                for px0 in (0, 4096):
                    for k in range(px0, px0 + 4096, 512):
                        nc.tensor.matmul(
                            ps[:, k - px0:k - px0 + 512],
                            wq, blueb[:, k:k + 512],
                            start=True, stop=(li == 0))
                    if li == 1:
                        for k in range(px0, px0 + 4096, 512):
                            nc.tensor.matmul(
                                ps[:, k - px0:k - px0 + 512],
                                wqg, T2[:, k:k + 512],
                                start=False, stop=True,
                                skip_group_check=True)
                    nc.scalar.activation(
                        T1[:, px0:px0 + 4096], ps[:],
                        ACTF.Identity, bias=qb)

            # prologue: layer-0 convs (drain order K, Q, V: logits-di0
            # needs K rows 0:64 + Q; V only needed at combine)
            emit_kv_conv(0, "k")
            emit_q_conv(0)
            emit_kv_conv(0, "v")

            loop = tc.For_i(0, reps, 1) if reps > 1 else contextlib.nullcontext()
            with loop:
                for li in range(NUM_LAYERS):
                    lwg = vecs[:, li, 3:4]

                    # ---- logits: S[3di:3di+3] = blocksum(Q * shift(K))
                    Qv = T1[:].rearrange("c (h w) -> c h w", w=W)
                    for di in range(3):
                        qb3 = Qv.unsqueeze(1).broadcast_to([C, 3, ROWS, W])
                        kb3 = _restride(
                            Kt[:, 4 * di:4 * di + ROWS, 0:W]
                            .unsqueeze(1).broadcast_to([C, 3, ROWS, W]),
                            1, TS)
                        p3v = big[:].rearrange("c n (h w) -> c n h w", w=W)
                        nc.vector.tensor_mul(p3v, qb3, kb3)
                        v = p3v.rearrange(
                            "c dj (th r) (tw s) -> c dj th tw r s",
                            r=TS, s=TS).rearrange(
                            "c dj th tw r s -> c (dj th) tw r s")
                        nc.vector.reduce_sum(
                            S[:, 3 * di:3 * di + 3, :], v, axis=AX.XY)

                    # ---- softmax over 9 neighbors (logits O(0.3), no max)
                    sf = S[:].rearrange("c n t -> c (n t)")
                    nc.scalar.activation(sf, sf, ACTF.Exp, scale=SCALE)
                    nc.vector.reduce_sum(
                        den[:], S[:].rearrange("c n t -> c t n"), axis=AX.X)
                    nc.vector.reciprocal(den[:], den[:])
                    # divide S by den per-slice so upsample di=0 starts
                    # before the full S is scaled
                    db = den[:].unsqueeze(1)
                    nc.vector.tensor_mul(
                        S[:, 0:3], S[:, 0:3], db.broadcast_to([C, 3, NTOK]))
                    nc.vector.tensor_mul(
                        S[:, 3:9], S[:, 3:9], db.broadcast_to([C, 6, NTOK]))

                    # ---- combine: O = sum_n upsample(A_n) * shift_n(V)
                    O = T2[:]
                    for di in range(3):
                        src = S[:, 3 * di:3 * di + 3, :].rearrange(
                            "c n (th tw) -> c n th tw", tw=NTW).unsqueeze(
                            4).broadcast_to([C, 3, NTH, NTW, TS])
                        nc.vector.tensor_copy(axp[:], src)
                        t3v = big[:].rearrange(
                            "c n (th r x) -> c n th r x", r=TS, x=W)
                        vap = _restride(
                            Vt[:, 4 * di:4 * di + ROWS, 0:W]
                            .unsqueeze(1).broadcast_to([C, 3, ROWS, W]),
                            1, TS).rearrange(
                            "c n (th r) x -> c n th r x", r=TS)
                        aap = axp[:].rearrange(
                            "c n th tw s -> c n th (tw s)").unsqueeze(
                            3).broadcast_to([C, 3, NTH, TS, W])
                        nc.vector.tensor_mul(t3v, vap, aap)
                        if di == 0:
                            nc.vector.tensor_add(O, big[:, 0, :], big[:, 1, :])
                            nc.vector.tensor_add(O, O, big[:, 2, :])
                        else:
                            for n in range(3):
                                nc.vector.tensor_add(O, O, big[:, n, :])

                    # ---- next layer's K conv: PE+ACT work that hides
                    # under this layer's LN DVE stream
                    nli = 1 - li
                    emit_next = (li == 0) or reps > 1
                    if emit_next:
                        emit_kv_conv(nli, "k")
                        emit_kv_conv(nli, "v")

                    # ---- LayerNorm over C via ones-matmul stats
                    o2 = big[:, 0, :]
                    mun = big[:, 1, :]
                    istd = big[:, 2, :]
                    nc.scalar.activation(o2, T2[:], ACTF.Square)
                    for (srcT, dstv, scl) in ((T2[:], mun, -1.0 / C),
                                              (o2, istd, 1.0 / C)):
                        for hx in (0, 4096):
                            for k in range(hx, hx + 4096, 512):
                                nc.tensor.matmul(
                                    ps[:, k - hx:k - hx + 512],
                                    ones[:], srcT[:, k:k + 512],
                                    start=True, stop=True)
                            nc.scalar.activation(
                                dstv[:, hx:hx + 4096], ps[:],
                                ACTF.Identity, scale=scl)
                    # istd holds E[x^2]; mun holds -mu
                    nc.scalar.activation(o2, mun, ACTF.Square)  # mu^2
                    nc.vector.tensor_add(T2[:], T2[:], mun)     # O - mu
                    nc.vector.tensor_sub(istd, istd, o2)        # var
                    nc.scalar.activation(istd, istd, ACTF.Ln, bias=eps_t[:])
                    nc.scalar.activation(istd, istd, ACTF.Exp, scale=-0.5)
                    nc.vector.tensor_mul(T2[:], T2[:], istd)    # N
                    if li == 0:
                        nc.vector.tensor_scalar(
                            acc[:], T2[:], lwg, c0, op0=ALU.mult, op1=ALU.add)
                    else:
                        nc.vector.scalar_tensor_tensor(
                            acc[:], T2[:], lwg, acc[:],
                            op0=ALU.mult, op1=ALU.add)
                    if emit_next:
                        emit_q_conv(nli)

            nc.sync.dma_start(d_out[:], acc[:])

    nc.compile()
    return nc


def _prep_inputs(blue, white, q_w, q_b, k_w, k_b, v_w, v_b, ln_g, ln_b,
                 layer_weights):
    bf16 = ml_dtypes.bfloat16
    f32 = np.float32

    blue = np.asarray(blue, f32)
    whiteP = np.zeros((B, C, H + 2 * TS, W), dtype=f32)
    whiteP[:, :, TS:TS + H, :] = np.asarray(white, f32)

    q_w = np.asarray(q_w, f32)
    q_b = np.asarray(q_b, f32)
    k_w = np.asarray(k_w, f32)
    v_w = np.asarray(v_w, f32)
    ln_b = np.asarray(ln_b, f32)
    ln_g = np.asarray(ln_g, f32)
    lwv = np.asarray(layer_weights, f32)

    wpack = np.zeros((C, NUM_LAYERS, 4 * C), dtype=bf16)
    for li in range(NUM_LAYERS):
        wpack[:, li, 0:C] = q_w[li].T.astype(bf16)
        wpack[:, li, C:2 * C] = k_w[li].T.astype(bf16)
        wpack[:, li, 2 * C:3 * C] = v_w[li].T.astype(bf16)
    # Q1 = Wq1@blue + (Wq1 diag(g0))@N0  (+ qb1 + Wq1@b0)
    wpack[:, 1, 3 * C:4 * C] = (q_w[1].T * ln_g[0][:, None]).astype(bf16)

    vecs = np.zeros((C, NUM_LAYERS, 4), dtype=f32)
    vecs[:, 0, 0] = q_b[0]
    vecs[:, 1, 0] = q_b[1] + q_w[1] @ ln_b[0]
    vecs[:, :, 1] = np.asarray(k_b, f32).T
    vecs[:, :, 2] = np.asarray(v_b, f32).T
    vecs[:, :, 3] = (ln_g * lwv.reshape(NUM_LAYERS, 1)).T

    in_maps = []
    for core in range(8):
        b, half = core // 2, core % 2
        y0 = half * ROWS
        consts = np.zeros((C, 4), f32)
        consts[:, 0] = 0.0 if half == 0 else 1.0
        consts[:, 1] = 0.0 if half == 1 else 1.0
        consts[:, 2] = ln_b[0] * lwv[0] + ln_b[1] * lwv[1]
        in_maps.append({
            "blueb": np.ascontiguousarray(
                blue[b, :, y0:y0 + ROWS, :]).reshape(C, NPIX).astype(bf16),
            "whiteb": np.ascontiguousarray(
                whiteP[b, :, y0:y0 + KROWS, :]).reshape(
                C, KROWS * W).astype(bf16),
            "w": wpack.reshape(C, NUM_LAYERS * 4 * C),
            "vecs": vecs.reshape(C, NUM_LAYERS * 4),
            "consts": consts,
        })
    return in_maps


def kernel(**inputs):
    from concourse.bass_utils import run_bass_kernel_spmd

    reps = int(os.environ.get("KBENCH_REPS", "1"))
    masked = bool(
        np.any(np.asarray(inputs["k_b"])) or np.any(np.asarray(inputs["v_b"])))
    key = ("nc", reps, masked)
    if key not in _CACHE:
        _CACHE[key] = _build(reps, masked)
    nc = _CACHE[key]

    in_maps = _prep_inputs(**inputs)
    res = run_bass_kernel_spmd(nc, in_maps, core_ids=list(range(8)))

    blue = np.asarray(inputs["blue"], np.float32)
    out = np.empty((B, C, H, W), np.float32)
    for core in range(8):
        b, half = core // 2, core % 2
        y0 = half * ROWS
        enh = np.asarray(res.results[core]["out"],
                         np.float32).reshape(C, ROWS, W)
        out[b, :, y0:y0 + ROWS, :] = blue[b, :, y0:y0 + ROWS, :] + enh
    return out
